# revision 3
# baseline (speedup 1.0000x reference)
"""Trainium2 Bass kernel for nn_MessageGNN (gnn_message_passing) — v2.

Destination-sharded edges across 8 cores (core k owns clauses
[k*50000,(k+1)*50000) and vars [k*12500,(k+1)*12500) plus every edge whose
destination falls in its slice), so segment sums are fully core-local.

One SPMD program (identical instruction stream on all 8 cores, per-core
data) dispatched with a single jit(shard_map) call — per-core schedule
constants (tiles per window, one-hot slice envelopes) are maxed/unioned
across cores on the host so the program is core-independent.

Per window of 1024 destinations:
  - src embeddings arrive as a host pre-gathered fp16 stream (edge-slot
    order), loaded feature-major with one HWDGE xbar-transpose DMA.
  - Edge MLP: per 128-edge tile, x^T / sat^T stationary against Wemb/Wsat,
    accumulating m[e,d] in PSUM; lrelu via ACT Prelu(alpha=.1) or DVE
    (0.1*x max x), alternating to balance engines.
  - Segment-mean via one-hot matmul: one-hot built by a two-scalar DVE op
    (iota == dst) * (1/cnt) at the per-tile envelope width, accumulated
    into a [128,1024] window PSUM as h^T.
  - Node MLP fused per 512-node half: feats+bias / ctx one-hot (host-built,
    DMA'd) / h / emb weight chunks; outputs transposed on PE (f16) and
    DMA'd out; phase-3 partial sums accumulate in a persistent PSUM tile.
The 64-row ctx update finishes on host from the per-core partial sums.
"""

import sys

sys.path.insert(0, "/opt/trn_rl_repo")

import numpy as np

NV, NC, NU, E, D = 100000, 400000, 64, 1200000, 128
M = 8
CS, VS = NC // M, NV // M
WIN = 1024
P = 128
PAD_DST = 1408.0

F16 = np.float16
F32 = np.float32


def _ceil(a, b):
    return -(-a // b)


def _prep_side(src, dst, sat, n_dst, tab16, we):
    """Edge-side prep: shared schedule + per-core slot tables.

    Returns (sched, percore) where sched is core-independent and percore[k]
    holds gs (pre-gathered src rows), satp ([5,S]), dw ([128,2*T_total]).
    """
    nwin = _ceil(n_dst, WIN)
    cores = []
    counts = np.zeros((M, nwin), np.int64)
    for k in range(M):
        base = k * n_dst
        mask = (dst >= base) & (dst < base + n_dst)
        es = np.nonzero(mask)[0]
        dstl = (dst[es] - base).astype(np.int64)
        order = np.argsort(dstl, kind="stable")
        es, dstl = es[order], dstl[order]
        counts[k] = np.bincount(dstl // WIN, minlength=nwin)
        cores.append((es, dstl))
    Tw = np.maximum(1, _ceil(counts.max(0), P)).astype(np.int64)
    toff = np.concatenate([[0], np.cumsum(Tw)])
    T_total = int(Tw.sum())
    S = T_total * P
    soff = toff * P

    slot_dst = np.full((M, S), -1, np.int64)
    slot_eid = np.full((M, S), -1, np.int64)
    for k, (es, dstl) in enumerate(cores):
        start = 0
        for wi in range(nwin):
            n = int(counts[k, wi])
            sl = slice(soff[wi], soff[wi] + n)
            slot_dst[k, sl] = dstl[start:start + n] - wi * WIN
            slot_eid[k, sl] = es[start:start + n]
            start += n

    # per-tile envelope (base/width across all cores) + 512-boundary slices
    tiles = []  # flat list over (window, tile): dict(base, width, slices)
    win_of_tile = np.repeat(np.arange(nwin), Tw)
    for ti in range(T_total):
        sl = slice(ti * P, (ti + 1) * P)
        dv = slot_dst[:, sl]
        real = dv >= 0
        if real.any():
            lo, hi = int(dv[real].min()), int(dv[real].max())
        else:
            lo, hi = 0, 0
        b0 = (lo // 32) * 32
        wd = _ceil(hi + 1 - b0, 32) * 32
        slices = []
        for h in (0, 1):
            s = max(b0, h * 512)
            e = min(b0 + wd, (h + 1) * 512)
            if s < e:
                slices.append((h, s - h * 512, s - b0, e - s))
        tiles.append(dict(base=b0, width=wd, slices=slices))

    # per-core tables
    percore = []
    base_of_slot = np.array([tiles[ti]["base"] for ti in range(T_total)],
                            np.int64).repeat(P)
    for k in range(M):
        dv, ev = slot_dst[k], slot_eid[k]
        real = dv >= 0
        dst_rel = np.full(S, PAD_DST, F32)
        dst_rel[real] = (dv[real] - base_of_slot[real]).astype(F32)
        wslot = np.zeros(S, F32)
        wslot[real] = we[dst[ev[real]]]
        dw = np.zeros((P, 2 * T_total), F32)
        dw[:, 0::2] = dst_rel.reshape(T_total, P).T
        dw[:, 1::2] = wslot.reshape(T_total, P).T
        satp = np.zeros((5, S), F16)
        satp[:4, real] = sat[ev[real]].T.astype(F16)
        satp[4, real] = 1.0
        gs = np.zeros((S, D), F16)
        gs[real] = tab16[src[ev[real]]]
        percore.append(dict(gs=gs, satp=satp, dw=dw))

    sched = dict(nwin=nwin, Tw=Tw.tolist(), toff=toff.tolist(),
                 soff=soff.tolist(), tiles=tiles, S=S, T_total=T_total,
                 Tmax=int(Tw.max()), Smax=int(Tw.max() * P))
    return sched, percore


def _prep_nodes(feats, emb16, ctx_ids, n_nodes, nwin):
    """Node tables for one core: fe [81, nwin*WIN], embl [128, nwin*WIN],
    ohj [128, nwin*512] (per-128-block ctx one-hot, u columns)."""
    Np = nwin * WIN
    fe = np.zeros((81, Np), F16)
    fe[ctx_ids, np.arange(n_nodes)] = 1.0          # rows 0:64 ctx one-hot
    fe[64:80, :n_nodes] = feats.T.astype(F16)      # rows 64:80 feats
    fe[80, :n_nodes] = 1.0                         # row 80 bias ones
    embl = np.zeros((P, Np), F16)
    embl[:, :n_nodes] = emb16.T
    nblk = nwin * 8
    ohj = np.zeros((P, nblk * 64), F16)
    node = np.arange(n_nodes)
    blk = node // P
    prow = node % P
    ohj[prow, blk * 64 + ctx_ids] = 1.0
    return fe, embl, ohj


def _build(sa, sb, nwinC, nwinV):
    import concourse.mybir as mybir
    import concourse.tile as tile
    from concourse import bacc
    from concourse.masks import make_identity

    f16, f32, i32 = mybir.dt.float16, mybir.dt.float32, mybir.dt.int32
    AF = mybir.ActivationFunctionType
    OP = mybir.AluOpType

    nc = bacc.Bacc("TRN2", target_bir_lowering=False, debug=False,
                   num_devices=1)
    io = {}

    def dram(name, shape, dt, kind="ExternalInput"):
        io[name] = nc.dram_tensor(name, list(shape), dt, kind=kind)
        return io[name]

    for side, s in (("A", sa), ("B", sb)):
        dram(f"gs{side}", [s["S"], D], f16)
        dram(f"satp{side}", [5, s["S"]], f16)
        dram(f"dw{side}", [P, 2 * s["T_total"]], f32)
        dram(f"Wemb{side}", [P, D], f16)
        dram(f"Wsat{side}", [5, D], f16)
    for sd, nwin in (("C", nwinC), ("V", nwinV)):
        dram(f"fe{sd}", [81, nwin * WIN], f16)
        dram(f"embl{sd}", [P, nwin * WIN], f16)
        dram(f"ohj{sd}", [P, nwin * 512], f16)
        dram(f"Wf{sd}", [P, D], f16)
        dram(f"Wh{sd}", [P, D], f16)
        dram(f"We{sd}", [P, D], f16)
        dram(f"ctxproj{sd}", [64, D], f16)
        dram(f"out{sd}", [nwin * 8, P, D], f16, kind="ExternalOutput")
    dram("acc", [P, P], f32, kind="ExternalOutput")

    with tile.TileContext(nc) as tc:
        with tc.tile_pool(name="const", bufs=1) as cpool, \
             tc.tile_pool(name="stage", bufs=2) as spool, \
             tc.tile_pool(name="meta", bufs=2) as mpool, \
             tc.tile_pool(name="work", bufs=3) as wpool, \
             tc.tile_pool(name="node", bufs=2) as npool, \
             tc.tile_pool(name="psE", bufs=2, space="PSUM") as psE, \
             tc.tile_pool(name="psH", bufs=1, space="PSUM") as psH, \
             tc.tile_pool(name="psN", bufs=2, space="PSUM") as psN, \
             tc.tile_pool(name="psT", bufs=1, space="PSUM") as psT, \
             tc.tile_pool(name="psAcc", bufs=1, space="PSUM") as psA:

            identF = cpool.tile([P, P], f16)
            make_identity(nc, identF[:])
            iota_i = cpool.tile([P, WIN], i32)
            nc.gpsimd.iota(iota_i[:], pattern=[[1, WIN]], base=0,
                           channel_multiplier=0)
            iota16 = cpool.tile([P, WIN], f16)
            nc.vector.tensor_copy(iota16[:], iota_i[:])
            z128 = cpool.tile([1, P], f16)
            nc.gpsimd.memset(z128[:], 0.0)
            z512 = cpool.tile([1, 512], f16)
            nc.gpsimd.memset(z512[:], 0.0)

            wt = {}
            for nm in ("WembA", "WsatA", "WembB", "WsatB",
                       "WfC", "WhC", "WeC", "ctxprojC",
                       "WfV", "WhV", "WeV", "ctxprojV"):
                t = cpool.tile(list(io[nm].shape), f16, tag=nm)
                nc.sync.dma_start(t[:], io[nm][:])
                wt[nm] = t

            acc_ps = psA.tile([P, P], f32, name="accps")
            nc.tensor.matmul(acc_ps[:], lhsT=z128[:], rhs=z512[:, :P],
                             start=True, stop=False, skip_group_check=True)

            Smax = max(sa["Smax"], sb["Smax"])
            Tmax = max(sa["Tmax"], sb["Tmax"])

            for side, sd, s, nwin in (("A", "C", sa, nwinC),
                                      ("B", "V", sb, nwinV)):
                acccol = 0 if sd == "C" else 64
                first_aps = [True]
                for wi in range(nwin):
                    T = s["Tw"][wi]
                    t0, s0 = s["toff"][wi], s["soff"][wi]
                    slots = T * P
                    stage = spool.tile([P, Smax], f16, tag="stage")
                    nc.sync.dma_start(stage[:, :slots],
                                      io[f"gs{side}"][s0:s0 + slots, :],
                                      transpose=True)
                    satp = mpool.tile([5, Smax], f16, tag="satp")
                    nc.sync.dma_start(satp[:, :slots],
                                      io[f"satp{side}"][:, s0:s0 + slots])
                    dw = mpool.tile([P, 2 * Tmax], f32, tag="dw")
                    nc.scalar.dma_start(dw[:, :2 * T],
                                        io[f"dw{side}"][:, 2 * t0:2 * (t0 + T)])
                    hps = [psH.tile([P, 512], f32, tag=f"h{i}", name=f"hps{i}")
                           for i in range(2)]
                    for b in range(_ceil(T, 4)):
                        nt = min(4, T - b * 4)
                        mps = psE.tile([P, 512], f32, tag="mps")
                        for t4 in range(nt):
                            t = b * 4 + t4
                            nc.tensor.matmul(
                                mps[:, t4 * P:(t4 + 1) * P],
                                lhsT=stage[:, t * P:(t + 1) * P],
                                rhs=wt[f"Wemb{side}"][:],
                                start=True, stop=False)
                            nc.tensor.matmul(
                                mps[:, t4 * P:(t4 + 1) * P],
                                lhsT=satp[:, t * P:(t + 1) * P],
                                rhs=wt[f"Wsat{side}"][:],
                                start=False, stop=True)
                        if b == 0:
                            for i in range(2):
                                nc.tensor.matmul(hps[i][:], lhsT=z128[:],
                                                 rhs=z512[:], start=True,
                                                 stop=False,
                                                 skip_group_check=True)
                        msb = wpool.tile([P, 512], f16, tag="msb")
                        nc.scalar.activation(msb[:, :nt * P],
                                             mps[:, :nt * P],
                                             AF.Prelu, alpha=0.1)
                        for t4 in range(nt):
                            t = b * 4 + t4
                            tm = s["tiles"][t0 + t]
                            wd = tm["width"]
                            ohw = wpool.tile([P, WIN], f16, tag="ohw")
                            nc.vector.tensor_scalar(
                                out=ohw[:, :wd], in0=iota16[:, :wd],
                                scalar1=dw[:, 2 * t:2 * t + 1],
                                scalar2=dw[:, 2 * t + 1:2 * t + 2],
                                op0=OP.is_equal, op1=OP.mult)
                            for (h, colw, colo, ln) in tm["slices"]:
                                nc.tensor.matmul(
                                    hps[h][:, colw:colw + ln],
                                    lhsT=msb[:, t4 * P:(t4 + 1) * P],
                                    rhs=ohw[:, colo:colo + ln],
                                    start=False, stop=True,
                                    skip_group_check=True)
                    hT = npool.tile([P, WIN], f16, tag="hT")
                    nc.vector.tensor_copy(hT[:, :512], hps[0][:])
                    nc.scalar.activation(hT[:, 512:], hps[1][:], AF.Copy)

                    # ---- node phase for this window ----
                    cga = wi * WIN
                    fe = npool.tile([81, WIN], f16, tag="fe")
                    nc.scalar.dma_start(fe[:], io[f"fe{sd}"][:, cga:cga + WIN])
                    embl = npool.tile([P, WIN], f16, tag="embl")
                    nc.sync.dma_start(embl[:], io[f"embl{sd}"][:, cga:cga + WIN])
                    ohj = npool.tile([P, 512], f16, tag="ohj")
                    nc.scalar.dma_start(ohj[:],
                                        io[f"ohj{sd}"][:, wi * 512:(wi + 1) * 512])
                    for g in (0, 1):
                        g0 = g * 512
                        nps = psN.tile([P, 512], f32, tag="nps")
                        nc.tensor.matmul(nps[:], lhsT=wt[f"Wf{sd}"][64:81, :],
                                         rhs=fe[64:81, g0:g0 + 512],
                                         start=True, stop=False)
                        nc.tensor.matmul(nps[:], lhsT=wt[f"ctxproj{sd}"][:],
                                         rhs=fe[:64, g0:g0 + 512],
                                         start=False, stop=False)
                        nc.tensor.matmul(nps[:], lhsT=wt[f"Wh{sd}"][:],
                                         rhs=hT[:, g0:g0 + 512],
                                         start=False, stop=False)
                        nc.tensor.matmul(nps[:], lhsT=wt[f"We{sd}"][:],
                                         rhs=embl[:, g0:g0 + 512],
                                         start=False, stop=True)
                        nsb = wpool.tile([P, 512], f16, tag="nsb")
                        nc.scalar.activation(nsb[:], nps[:], AF.Prelu,
                                             alpha=0.1)
                        tps = psT.tile([P, 512], f16, tag="tps")
                        for j in range(4):
                            nc.tensor.matmul(tps[:, j * P:(j + 1) * P],
                                             lhsT=nsb[:, j * P:(j + 1) * P],
                                             rhs=identF[:],
                                             is_transpose=True,
                                             skip_group_check=True)
                        osb = wpool.tile([P, 512], f16, tag="osb")
                        nc.vector.tensor_copy(osb[:], tps[:])
                        blk0 = (wi * 2 + g) * 4
                        eng = nc.sync if g == 0 else nc.scalar
                        eng.dma_start(
                            io[f"out{sd}"][blk0:blk0 + 4, :, :]
                            .rearrange("j p d -> p j d"),
                            osb[:].rearrange("p (j d) -> p j d", j=4))
                        for j in range(4):
                            nc.tensor.matmul(
                                acc_ps[:, acccol:acccol + 64],
                                lhsT=osb[:, j * P:(j + 1) * P],
                                rhs=ohj[:, (g * 4 + j) * 64:(g * 4 + j + 1) * 64],
                                start=False, stop=False,
                                skip_group_check=True)
            nc.tensor.matmul(acc_ps[:], lhsT=z128[:], rhs=z512[:, :P],
                             start=False, stop=True, skip_group_check=True)
            accsb = cpool.tile([P, P], f32, tag="accsb")
            nc.vector.tensor_copy(accsb[:], acc_ps[:])
            nc.sync.dma_start(io["acc"][:], accsb[:])
    nc.compile()
    return nc


_timing_handles = []


def _run_spmd(nc, in_maps):
    """One jit(shard_map) dispatch running the SPMD program on 8 cores."""
    import jax
    from jax.sharding import Mesh, PartitionSpec, NamedSharding
    from jax.experimental.shard_map import shard_map
    from concourse.bass2jax import _bass_exec_p, install_neuronx_cc_hook
    import concourse.mybir as mybir

    install_neuronx_cc_hook()
    devs = jax.devices()[:M]

    in_names, out_names, out_avals, zero_outs = [], [], [], []
    pid_name = None
    for alloc in nc.m.functions[0].allocations:
        if not isinstance(alloc, mybir.MemoryLocationSet):
            continue
        name = alloc.memorylocations[0].name
        if alloc.kind == "ExternalInput":
            if name == "partition_id":
                pid_name = name
            else:
                in_names.append(name)
        elif alloc.kind == "ExternalOutput":
            shape = tuple(alloc.tensor_shape)
            dtype = mybir.dt.np(alloc.dtype)
            out_names.append(name)
            out_avals.append(jax.core.ShapedArray(shape, dtype))
            zero_outs.append(np.zeros(shape, dtype))
    n_params = len(in_names)
    all_names = list(in_names) + list(out_names)
    if pid_name:
        all_names.append(pid_name)

    def _body(*args, _oa=tuple(out_avals), _an=tuple(all_names),
              _on=tuple(out_names), _nc=nc):
        return tuple(_bass_exec_p.bind(
            *args, out_avals=_oa, in_names=_an, out_names=_on,
            lowering_input_output_aliases=(),
            sim_require_finite=True, sim_require_nnan=True, nc=_nc,
        ))

    mesh = Mesh(np.asarray(devs), ("core",))
    nops = n_params + len(zero_outs) + (1 if pid_name else 0)
    spec = PartitionSpec("core")
    fn = jax.jit(shard_map(_body, mesh=mesh, in_specs=(spec,) * nops,
                           out_specs=(spec,) * len(out_names),
                           check_rep=False), keep_unused=True)

    concat_in = [np.concatenate([np.asarray(m[nm]) for m in in_maps], axis=0)
                 for nm in in_names]
    concat_zeros = [np.zeros((M * z.shape[0], *z.shape[1:]), z.dtype)
                    for z in zero_outs]
    args = concat_in + concat_zeros
    if pid_name:
        args.append(np.arange(M, dtype=np.uint32).reshape(M, 1))

    out_arrs = fn(*args)
    res = [np.asarray(o) for o in out_arrs]
    _timing_handles.append(dict(fn=fn, args=args, devs=devs, mesh=mesh))
    return {nm: res[i].reshape(M, *out_avals[i].shape)
            for i, nm in enumerate(out_names)}


def kernel(**inputs):
    inp = {k: np.asarray(v) for k, v in inputs.items()}
    var_emb, clause_emb, ctx_emb = (inp["var_emb"], inp["clause_emb"],
                                    inp["ctx_emb"])
    nv, ncl, nu = var_emb.shape[0], clause_emb.shape[0], ctx_emb.shape[0]
    cs, vs = ncl // M, nv // M

    a_src = inp["assigns_src"].astype(np.int64)
    a_dst = inp["assigns_dst"].astype(np.int64)
    c_src = inp["contains_src"].astype(np.int64)
    c_dst = inp["contains_dst"].astype(np.int64)
    var_ctx = inp["var_ctx"].astype(np.int64)
    clause_ctx = inp["clause_ctx"].astype(np.int64)

    cnt_c = np.bincount(a_dst, minlength=ncl).astype(F32)
    cnt_v = np.bincount(c_dst, minlength=nv).astype(F32)
    we_c = 1.0 / np.maximum(cnt_c, 1.0)
    we_v = 1.0 / np.maximum(cnt_v, 1.0)

    var16 = var_emb.astype(F16)
    clause16 = clause_emb.astype(F16)

    W_vc, b_vc = inp["W_vc"].astype(F32), inp["b_vc"].astype(F32)
    W_cv, b_cv = inp["W_cv"].astype(F32), inp["b_cv"].astype(F32)

    sa, pcA = _prep_side(a_src, a_dst, inp["edge_sat_vc"], cs, var16, we_c)
    sb, pcB = _prep_side(c_src, c_dst, inp["edge_sat_cv"], vs, clause16, we_v)
    nwinC, nwinV = sa["nwin"], sb["nwin"]

    def node_w(Wn, bn):
        Wn, bn = Wn.astype(F32), bn.astype(F32)
        Wf = np.zeros((P, D), F16)
        Wf[64:80] = Wn[:16].astype(F16)
        Wf[80] = bn.astype(F16)
        Wh = np.ascontiguousarray(Wn[16:16 + D]).astype(F16)
        ctxproj = (ctx_emb.astype(F32) @ Wn[16 + D:16 + 2 * D]).astype(F16)
        We = np.ascontiguousarray(Wn[16 + 2 * D:16 + 3 * D]).astype(F16)
        return Wf, Wh, ctxproj, We

    WfC, WhC, ctxprojC, WeC = node_w(inp["W_c"], inp["b_c"])
    WfV, WhV, ctxprojV, WeV = node_w(inp["W_v"], inp["b_v"])

    common = dict(
        WembA=np.ascontiguousarray(W_vc[4:4 + D]).astype(F16),
        WsatA=np.vstack([W_vc[:4], b_vc[None, :]]).astype(F16),
        WembB=np.ascontiguousarray(W_cv[4:4 + D]).astype(F16),
        WsatB=np.vstack([W_cv[:4], b_cv[None, :]]).astype(F16),
        WfC=WfC, WhC=WhC, WeC=WeC, ctxprojC=ctxprojC,
        WfV=WfV, WhV=WhV, WeV=WeV, ctxprojV=ctxprojV,
    )

    in_maps = []
    for k in range(M):
        feC, emC, ohjC = _prep_nodes(
            inp["clause_feats"][k * cs:(k + 1) * cs],
            clause16[k * cs:(k + 1) * cs],
            clause_ctx[k * cs:(k + 1) * cs], cs, nwinC)
        feV, emV, ohjV = _prep_nodes(
            inp["var_feats"][k * vs:(k + 1) * vs],
            var16[k * vs:(k + 1) * vs],
            var_ctx[k * vs:(k + 1) * vs], vs, nwinV)
        in_maps.append(dict(
            gsA=pcA[k]["gs"], satpA=pcA[k]["satp"], dwA=pcA[k]["dw"],
            gsB=pcB[k]["gs"], satpB=pcB[k]["satp"], dwB=pcB[k]["dw"],
            feC=feC, emblC=emC, ohjC=ohjC,
            feV=feV, emblV=emV, ohjV=ohjV,
            **common,
        ))

    nc = _build(sa, sb, nwinC, nwinV)
    res = _run_spmd(nc, in_maps)

    new_clause = res["outC"].reshape(M, -1, D)[:, :cs].reshape(ncl, D)
    new_var = res["outV"].reshape(M, -1, D)[:, :vs].reshape(nv, D)
    acc = res["acc"].sum(0)  # [128 d, 128] cols 0:64 C, 64:128 V
    accC, accV = acc[:, :64], acc[:, 64:]

    cnt_cu = np.bincount(clause_ctx, minlength=nu).astype(F32)
    cnt_vu = np.bincount(var_ctx, minlength=nu).astype(F32)
    c_ctx = (accC / np.maximum(cnt_cu, 1.0)[None, :]).T
    v_ctx = (accV / np.maximum(cnt_vu, 1.0)[None, :]).T
    zu = np.concatenate([inp["ctx_feats"].astype(F32), c_ctx, v_ctx,
                         ctx_emb.astype(F32)], 1) @ inp["W_u"].astype(F32) \
        + inp["b_u"].astype(F32)
    new_ctx = np.where(zu >= 0, zu, 0.1 * zu).astype(F32)

    out = np.empty((ncl + nv + nu, D), F32)
    out[:ncl] = new_clause.astype(F32)
    out[ncl:ncl + nv] = new_var.astype(F32)
    out[ncl + nv:] = new_ctx
    return out


# revision 5
# speedup vs baseline: 1.2759x; 1.2759x over previous
"""Trainium2 Bass kernel for nn_MessageGNN (gnn_message_passing) — v2.

Destination-sharded edges across 8 cores (core k owns clauses
[k*50000,(k+1)*50000) and vars [k*12500,(k+1)*12500) plus every edge whose
destination falls in its slice), so segment sums are fully core-local.

One SPMD program (identical instruction stream on all 8 cores, per-core
data) dispatched with a single jit(shard_map) call — per-core schedule
constants (tiles per window, one-hot slice envelopes) are maxed/unioned
across cores on the host so the program is core-independent.

Per window of 1024 destinations:
  - src embeddings arrive as a host pre-gathered fp16 stream (edge-slot
    order), loaded feature-major with one HWDGE xbar-transpose DMA.
  - Edge MLP: per 128-edge tile, x^T / sat^T stationary against Wemb/Wsat,
    accumulating m[e,d] in PSUM; lrelu via ACT Prelu(alpha=.1) or DVE
    (0.1*x max x), alternating to balance engines.
  - Segment-mean via one-hot matmul: one-hot built by a two-scalar DVE op
    (iota == dst) * (1/cnt) at the per-tile envelope width, accumulated
    into a [128,1024] window PSUM as h^T.
  - Node MLP fused per 512-node half: feats+bias / ctx one-hot (host-built,
    DMA'd) / h / emb weight chunks; outputs transposed on PE (f16) and
    DMA'd out; phase-3 partial sums accumulate in a persistent PSUM tile.
The 64-row ctx update finishes on host from the per-core partial sums.
"""

import sys

sys.path.insert(0, "/opt/trn_rl_repo")

import numpy as np

NV, NC, NU, E, D = 100000, 400000, 64, 1200000, 128
M = 8
CS, VS = NC // M, NV // M
WIN = 1024
P = 128
PAD_DST = 1408.0

F16 = np.float16
F32 = np.float32


def _ceil(a, b):
    return -(-a // b)


def _prep_side(src, dst, sat, n_dst, tab16, we):
    """Edge-side prep: shared schedule + per-core slot tables.

    Returns (sched, percore) where sched is core-independent and percore[k]
    holds gs (pre-gathered src rows), satp ([5,S]), dw ([128,2*T_total]).
    """
    nwin = _ceil(n_dst, WIN)
    cores = []
    counts = np.zeros((M, nwin), np.int64)
    for k in range(M):
        base = k * n_dst
        mask = (dst >= base) & (dst < base + n_dst)
        es = np.nonzero(mask)[0]
        dstl = (dst[es] - base).astype(np.int64)
        order = np.argsort(dstl, kind="stable")
        es, dstl = es[order], dstl[order]
        counts[k] = np.bincount(dstl // WIN, minlength=nwin)
        cores.append((es, dstl))
    Tw = np.maximum(1, _ceil(counts.max(0), P)).astype(np.int64)
    toff = np.concatenate([[0], np.cumsum(Tw)])
    T_total = int(Tw.sum())
    S = T_total * P
    soff = toff * P

    slot_dst = np.full((M, S), -1, np.int64)
    slot_eid = np.full((M, S), -1, np.int64)
    for k, (es, dstl) in enumerate(cores):
        start = 0
        for wi in range(nwin):
            n = int(counts[k, wi])
            sl = slice(soff[wi], soff[wi] + n)
            slot_dst[k, sl] = dstl[start:start + n] - wi * WIN
            slot_eid[k, sl] = es[start:start + n]
            start += n

    # per-tile envelope (base/width across all cores) + 512-boundary slices
    tiles = []  # flat list over (window, tile): dict(base, width, slices)
    win_of_tile = np.repeat(np.arange(nwin), Tw)
    for ti in range(T_total):
        sl = slice(ti * P, (ti + 1) * P)
        dv = slot_dst[:, sl]
        real = dv >= 0
        if real.any():
            lo, hi = int(dv[real].min()), int(dv[real].max())
        else:
            lo, hi = 0, 0
        b0 = (lo // 32) * 32
        wd = _ceil(hi + 1 - b0, 32) * 32
        slices = []
        for h in (0, 1):
            s = max(b0, h * 512)
            e = min(b0 + wd, (h + 1) * 512)
            if s < e:
                slices.append((h, s - h * 512, s - b0, e - s))
        tiles.append(dict(base=b0, width=wd, slices=slices))

    # per-core tables
    percore = []
    base_of_slot = np.array([tiles[ti]["base"] for ti in range(T_total)],
                            np.int64).repeat(P)
    for k in range(M):
        dv, ev = slot_dst[k], slot_eid[k]
        real = dv >= 0
        dst_rel = np.full(S, PAD_DST, F32)
        dst_rel[real] = (dv[real] - base_of_slot[real]).astype(F32)
        wslot = np.zeros(S, F32)
        wslot[real] = we[dst[ev[real]]]
        dw = np.zeros((P, 2 * T_total), F32)
        dw[:, 0::2] = dst_rel.reshape(T_total, P).T
        dw[:, 1::2] = wslot.reshape(T_total, P).T
        satp = np.zeros((5, S), F16)
        satp[:4, real] = sat[ev[real]].T.astype(F16)
        satp[4, real] = 1.0
        gs = np.zeros((S, D), F16)
        gs[real] = tab16[src[ev[real]]]
        percore.append(dict(gs=gs, satp=satp, dw=dw))

    sched = dict(nwin=nwin, Tw=Tw.tolist(), toff=toff.tolist(),
                 soff=soff.tolist(), tiles=tiles, S=S, T_total=T_total,
                 Tmax=int(Tw.max()), Smax=int(Tw.max() * P))
    return sched, percore


def _prep_nodes(feats, emb16, ctx_ids, n_nodes, nwin):
    """Node tables for one core: fe [81, nwin*WIN], embl [128, nwin*WIN],
    ohj [128, nwin*512] (per-128-block ctx one-hot, u columns)."""
    Np = nwin * WIN
    fe = np.zeros((81, Np), F16)
    fe[ctx_ids, np.arange(n_nodes)] = 1.0          # rows 0:64 ctx one-hot
    fe[64:80, :n_nodes] = feats.T.astype(F16)      # rows 64:80 feats
    fe[80, :n_nodes] = 1.0                         # row 80 bias ones
    embl = np.zeros((P, Np), F16)
    embl[:, :n_nodes] = emb16.T
    nblk = nwin * 8
    ohj = np.zeros((P, nblk * 64), F16)
    node = np.arange(n_nodes)
    blk = node // P
    prow = node % P
    ohj[prow, blk * 64 + ctx_ids] = 1.0
    return fe, embl, ohj


def _build(sa, sb, nwinC, nwinV):
    import concourse.mybir as mybir
    import concourse.tile as tile
    from concourse import bacc
    from concourse.masks import make_identity

    f16, f32, i32 = mybir.dt.float16, mybir.dt.float32, mybir.dt.int32
    AF = mybir.ActivationFunctionType
    OP = mybir.AluOpType

    nc = bacc.Bacc("TRN2", target_bir_lowering=False, debug=False,
                   num_devices=1)
    io = {}

    def dram(name, shape, dt, kind="ExternalInput"):
        io[name] = nc.dram_tensor(name, list(shape), dt, kind=kind)
        return io[name]

    for side, s in (("A", sa), ("B", sb)):
        dram(f"gs{side}", [s["S"], D], f16)
        dram(f"satp{side}", [5, s["S"]], f16)
        dram(f"dw{side}", [P, 2 * s["T_total"]], f32)
        dram(f"Wemb{side}", [P, D], f16)
        dram(f"Wsat{side}", [5, D], f16)
    for sd, nwin in (("C", nwinC), ("V", nwinV)):
        dram(f"fe{sd}", [81, nwin * WIN], f16)
        dram(f"embl{sd}", [P, nwin * WIN], f16)
        dram(f"ohj{sd}", [P, nwin * 512], f16)
        dram(f"Wf{sd}", [P, D], f16)
        dram(f"Wh{sd}", [P, D], f16)
        dram(f"We{sd}", [P, D], f16)
        dram(f"ctxproj{sd}", [64, D], f16)
        dram(f"out{sd}", [nwin * 8, P, D], f16, kind="ExternalOutput")
    dram("acc", [P, P], f32, kind="ExternalOutput")

    with tile.TileContext(nc) as tc:
        with tc.tile_pool(name="const", bufs=1) as cpool, \
             tc.tile_pool(name="stage", bufs=2) as spool, \
             tc.tile_pool(name="meta", bufs=2) as mpool, \
             tc.tile_pool(name="work", bufs=3) as wpool, \
             tc.tile_pool(name="oh", bufs=10) as opool, \
             tc.tile_pool(name="node", bufs=2) as npool, \
             tc.tile_pool(name="psE", bufs=2, space="PSUM") as psE, \
             tc.tile_pool(name="psH", bufs=1, space="PSUM") as psH, \
             tc.tile_pool(name="psN", bufs=2, space="PSUM") as psN, \
             tc.tile_pool(name="psT", bufs=1, space="PSUM") as psT, \
             tc.tile_pool(name="psAcc", bufs=1, space="PSUM") as psA:

            identF = cpool.tile([P, P], f16)
            make_identity(nc, identF[:])
            iota_i = cpool.tile([P, WIN], i32)
            nc.gpsimd.iota(iota_i[:], pattern=[[1, WIN]], base=0,
                           channel_multiplier=0)
            iota16 = cpool.tile([P, WIN], f16)
            nc.vector.tensor_copy(iota16[:], iota_i[:])
            z128 = cpool.tile([1, P], f16)
            nc.gpsimd.memset(z128[:], 0.0)
            z512 = cpool.tile([1, 512], f16)
            nc.gpsimd.memset(z512[:], 0.0)

            wt = {}
            for nm in ("WembA", "WsatA", "WembB", "WsatB",
                       "WfC", "WhC", "WeC", "ctxprojC",
                       "WfV", "WhV", "WeV", "ctxprojV"):
                t = cpool.tile(list(io[nm].shape), f16, tag=nm)
                nc.sync.dma_start(t[:], io[nm][:])
                wt[nm] = t

            acc_ps = psA.tile([P, P], f32, name="accps")
            nc.tensor.matmul(acc_ps[:], lhsT=z128[:], rhs=z512[:, :P],
                             start=True, stop=False, skip_group_check=True)

            Smax = max(sa["Smax"], sb["Smax"])
            Tmax = max(sa["Tmax"], sb["Tmax"])

            def issue_edge(side, s, wi):
                T = s["Tw"][wi]
                t0, s0 = s["toff"][wi], s["soff"][wi]
                slots = T * P
                stage = spool.tile([P, Smax], f16, tag="stage")
                nc.sync.dma_start(stage[:, :slots],
                                  io[f"gs{side}"][s0:s0 + slots, :],
                                  transpose=True)
                satp = mpool.tile([5, Smax], f16, tag="satp")
                nc.scalar.dma_start(satp[:, :slots],
                                    io[f"satp{side}"][:, s0:s0 + slots])
                dw = mpool.tile([P, 2 * Tmax], f32, tag="dw")
                nc.scalar.dma_start(dw[:, :2 * T],
                                    io[f"dw{side}"][:, 2 * t0:2 * (t0 + T)])
                return dict(stage=stage, satp=satp, dw=dw, T=T, t0=t0)

            def issue_node(sd, wi):
                cga = wi * WIN
                fe = npool.tile([81, WIN], f16, tag="fe")
                nc.scalar.dma_start(fe[:], io[f"fe{sd}"][:, cga:cga + WIN])
                embl = npool.tile([P, WIN], f16, tag="embl")
                nc.sync.dma_start(embl[:], io[f"embl{sd}"][:, cga:cga + WIN])
                ohj = npool.tile([P, 512], f16, tag="ohj")
                nc.scalar.dma_start(ohj[:],
                                    io[f"ohj{sd}"][:, wi * 512:(wi + 1) * 512])
                return dict(fe=fe, embl=embl, ohj=ohj)

            jobs = [("A", "C", sa, wi) for wi in range(nwinC)] + \
                   [("B", "V", sb, wi) for wi in range(nwinV)]
            pf_e = issue_edge(jobs[0][0], jobs[0][2], jobs[0][3])
            pf_n = issue_node(jobs[0][1], jobs[0][3])
            for ji, (side, sd, s, wi) in enumerate(jobs):
                acccol = 0 if sd == "C" else 64
                cur_e, cur_n = pf_e, pf_n
                if ji + 1 < len(jobs):
                    nside, nsd, ns, nwi = jobs[ji + 1]
                    pf_e = issue_edge(nside, ns, nwi)
                    pf_n = issue_node(nsd, nwi)
                if True:
                    T, t0 = cur_e["T"], cur_e["t0"]
                    stage, satp, dw = (cur_e["stage"], cur_e["satp"],
                                       cur_e["dw"])
                    hps = [psH.tile([P, 512], f32, tag=f"h{i}", name=f"hps{i}")
                           for i in range(2)]
                    # last block touching each window half (for early hT copy)
                    bsplit = [0, 0]
                    for t in range(T):
                        for (h, _cw, _co, _ln) in s["tiles"][t0 + t]["slices"]:
                            bsplit[h] = max(bsplit[h], t // 4)
                    hT = npool.tile([P, WIN], f16, tag="hT")
                    def emit_oh(pend):
                        msb_p, b_p, nt_p, ohws_p = pend
                        for t4 in range(nt_p):
                            t = b_p * 4 + t4
                            tm = s["tiles"][t0 + t]
                            for (h, colw, colo, ln) in tm["slices"]:
                                nc.tensor.matmul(
                                    hps[h][:, colw:colw + ln],
                                    lhsT=msb_p[:, t4 * P:(t4 + 1) * P],
                                    rhs=ohws_p[t4][:, colo:colo + ln],
                                    start=False, stop=True,
                                    skip_group_check=True)

                    def emit_hcopy(b_done):
                        if b_done == bsplit[0]:
                            nc.vector.tensor_copy(hT[:, :512], hps[0][:])
                        if b_done == bsplit[1]:
                            nc.scalar.activation(hT[:, 512:], hps[1][:],
                                                 AF.Copy)

                    pend = None
                    for b in range(_ceil(T, 4)):
                        nt = min(4, T - b * 4)
                        mps = psE.tile([P, 512], f32, tag="mps")
                        for t4 in range(nt):
                            t = b * 4 + t4
                            nc.tensor.matmul(
                                mps[:, t4 * P:(t4 + 1) * P],
                                lhsT=stage[:, t * P:(t + 1) * P],
                                rhs=wt[f"Wemb{side}"][:],
                                start=True, stop=False)
                            nc.tensor.matmul(
                                mps[:, t4 * P:(t4 + 1) * P],
                                lhsT=satp[:, t * P:(t + 1) * P],
                                rhs=wt[f"Wsat{side}"][:],
                                start=False, stop=True)
                        if b == 0:
                            for i in range(2):
                                nc.tensor.matmul(hps[i][:], lhsT=z128[:],
                                                 rhs=z512[:], start=True,
                                                 stop=False,
                                                 skip_group_check=True)
                        ohws = []
                        for t4 in range(nt):
                            t = b * 4 + t4
                            tm = s["tiles"][t0 + t]
                            wd = tm["width"]
                            ohw = opool.tile([P, WIN], f16, tag="ohw")
                            nc.vector.tensor_scalar(
                                out=ohw[:, :wd], in0=iota16[:, :wd],
                                scalar1=dw[:, 2 * t:2 * t + 1],
                                scalar2=dw[:, 2 * t + 1:2 * t + 2],
                                op0=OP.is_equal, op1=OP.mult)
                            ohws.append(ohw)
                        msb = wpool.tile([P, 512], f16, tag="msb")
                        nc.scalar.activation(msb[:, :nt * P],
                                             mps[:, :nt * P],
                                             AF.Prelu, alpha=0.1)
                        if pend is not None:
                            emit_oh(pend)
                            emit_hcopy(pend[1])
                        pend = (msb, b, nt, ohws)
                    if pend is not None:
                        emit_oh(pend)
                        emit_hcopy(pend[1])

                    # ---- node phase for this window ----
                    fe, embl, ohj = cur_n["fe"], cur_n["embl"], cur_n["ohj"]
                    for g in (0, 1):
                        g0 = g * 512
                        nps = psN.tile([P, 512], f32, tag="nps")
                        nc.tensor.matmul(nps[:], lhsT=wt[f"Wf{sd}"][64:81, :],
                                         rhs=fe[64:81, g0:g0 + 512],
                                         start=True, stop=False)
                        nc.tensor.matmul(nps[:], lhsT=wt[f"ctxproj{sd}"][:],
                                         rhs=fe[:64, g0:g0 + 512],
                                         start=False, stop=False)
                        nc.tensor.matmul(nps[:], lhsT=wt[f"Wh{sd}"][:],
                                         rhs=hT[:, g0:g0 + 512],
                                         start=False, stop=False)
                        nc.tensor.matmul(nps[:], lhsT=wt[f"We{sd}"][:],
                                         rhs=embl[:, g0:g0 + 512],
                                         start=False, stop=True)
                        nsb = wpool.tile([P, 512], f16, tag="nsb")
                        nc.scalar.activation(nsb[:], nps[:], AF.Prelu,
                                             alpha=0.1)
                        tps = psT.tile([P, 512], f16, tag="tps")
                        for j in range(4):
                            nc.tensor.matmul(tps[:, j * P:(j + 1) * P],
                                             lhsT=nsb[:, j * P:(j + 1) * P],
                                             rhs=identF[:],
                                             is_transpose=True,
                                             skip_group_check=True)
                        osb = wpool.tile([P, 512], f16, tag="osb")
                        nc.vector.tensor_copy(osb[:], tps[:])
                        blk0 = (wi * 2 + g) * 4
                        eng = nc.sync if g == 0 else nc.scalar
                        eng.dma_start(
                            io[f"out{sd}"][blk0:blk0 + 4, :, :]
                            .rearrange("j p d -> p j d"),
                            osb[:].rearrange("p (j d) -> p j d", j=4))
                        for j in range(4):
                            nc.tensor.matmul(
                                acc_ps[:, acccol:acccol + 64],
                                lhsT=osb[:, j * P:(j + 1) * P],
                                rhs=ohj[:, (g * 4 + j) * 64:(g * 4 + j + 1) * 64],
                                start=False, stop=False,
                                skip_group_check=True)
            nc.tensor.matmul(acc_ps[:], lhsT=z128[:], rhs=z512[:, :P],
                             start=False, stop=True, skip_group_check=True)
            accsb = cpool.tile([P, P], f32, tag="accsb")
            nc.vector.tensor_copy(accsb[:], acc_ps[:])
            nc.sync.dma_start(io["acc"][:], accsb[:])
    nc.compile()
    return nc


_timing_handles = []


def _run_spmd(nc, in_maps):
    """One jit(shard_map) dispatch running the SPMD program on 8 cores."""
    import jax
    from jax.sharding import Mesh, PartitionSpec, NamedSharding
    from jax.experimental.shard_map import shard_map
    from concourse.bass2jax import _bass_exec_p, install_neuronx_cc_hook
    import concourse.mybir as mybir

    install_neuronx_cc_hook()
    devs = jax.devices()[:M]

    in_names, out_names, out_avals, zero_outs = [], [], [], []
    pid_name = None
    for alloc in nc.m.functions[0].allocations:
        if not isinstance(alloc, mybir.MemoryLocationSet):
            continue
        name = alloc.memorylocations[0].name
        if alloc.kind == "ExternalInput":
            if name == "partition_id":
                pid_name = name
            else:
                in_names.append(name)
        elif alloc.kind == "ExternalOutput":
            shape = tuple(alloc.tensor_shape)
            dtype = mybir.dt.np(alloc.dtype)
            out_names.append(name)
            out_avals.append(jax.core.ShapedArray(shape, dtype))
            zero_outs.append(np.zeros(shape, dtype))
    n_params = len(in_names)
    all_names = list(in_names) + list(out_names)
    if pid_name:
        all_names.append(pid_name)

    def _body(*args, _oa=tuple(out_avals), _an=tuple(all_names),
              _on=tuple(out_names), _nc=nc):
        return tuple(_bass_exec_p.bind(
            *args, out_avals=_oa, in_names=_an, out_names=_on,
            lowering_input_output_aliases=(),
            sim_require_finite=True, sim_require_nnan=True, nc=_nc,
        ))

    mesh = Mesh(np.asarray(devs), ("core",))
    nops = n_params + len(zero_outs) + (1 if pid_name else 0)
    spec = PartitionSpec("core")
    fn = jax.jit(shard_map(_body, mesh=mesh, in_specs=(spec,) * nops,
                           out_specs=(spec,) * len(out_names),
                           check_rep=False), keep_unused=True)

    concat_in = [np.concatenate([np.asarray(m[nm]) for m in in_maps], axis=0)
                 for nm in in_names]
    concat_zeros = [np.zeros((M * z.shape[0], *z.shape[1:]), z.dtype)
                    for z in zero_outs]
    args = concat_in + concat_zeros
    if pid_name:
        args.append(np.arange(M, dtype=np.uint32).reshape(M, 1))

    def make_loop_fn(K):
        """jit that executes the kernel K times back-to-back on-device
        (scan carries the output buffers), for dispatch-free timing."""
        n_outs = len(out_names)

        def _body_k(*args):
            data = args[:n_params]
            zeros = tuple(args[n_params:n_params + n_outs])
            pid = args[-1] if pid_name else None

            def step(carry, _):
                operands = list(data) + list(carry)
                if pid is not None:
                    operands.append(pid)
                outs = _bass_exec_p.bind(
                    *operands, out_avals=tuple(out_avals),
                    in_names=tuple(all_names), out_names=tuple(out_names),
                    lowering_input_output_aliases=(),
                    sim_require_finite=True, sim_require_nnan=True, nc=nc)
                return tuple(outs), None

            carry, _ = jax.lax.scan(step, zeros, None, length=K)
            return carry

        return jax.jit(shard_map(_body_k, mesh=mesh, in_specs=(spec,) * nops,
                                 out_specs=(spec,) * len(out_names),
                                 check_rep=False), keep_unused=True)

    out_arrs = fn(*args)
    res = [np.asarray(o) for o in out_arrs]
    _timing_handles.append(dict(fn=fn, args=args, devs=devs, mesh=mesh,
                                make_loop_fn=make_loop_fn))
    return {nm: res[i].reshape(M, *out_avals[i].shape)
            for i, nm in enumerate(out_names)}


def kernel(**inputs):
    inp = {k: np.asarray(v) for k, v in inputs.items()}
    var_emb, clause_emb, ctx_emb = (inp["var_emb"], inp["clause_emb"],
                                    inp["ctx_emb"])
    nv, ncl, nu = var_emb.shape[0], clause_emb.shape[0], ctx_emb.shape[0]
    cs, vs = ncl // M, nv // M

    a_src = inp["assigns_src"].astype(np.int64)
    a_dst = inp["assigns_dst"].astype(np.int64)
    c_src = inp["contains_src"].astype(np.int64)
    c_dst = inp["contains_dst"].astype(np.int64)
    var_ctx = inp["var_ctx"].astype(np.int64)
    clause_ctx = inp["clause_ctx"].astype(np.int64)

    cnt_c = np.bincount(a_dst, minlength=ncl).astype(F32)
    cnt_v = np.bincount(c_dst, minlength=nv).astype(F32)
    we_c = 1.0 / np.maximum(cnt_c, 1.0)
    we_v = 1.0 / np.maximum(cnt_v, 1.0)

    var16 = var_emb.astype(F16)
    clause16 = clause_emb.astype(F16)

    W_vc, b_vc = inp["W_vc"].astype(F32), inp["b_vc"].astype(F32)
    W_cv, b_cv = inp["W_cv"].astype(F32), inp["b_cv"].astype(F32)

    sa, pcA = _prep_side(a_src, a_dst, inp["edge_sat_vc"], cs, var16, we_c)
    sb, pcB = _prep_side(c_src, c_dst, inp["edge_sat_cv"], vs, clause16, we_v)
    nwinC, nwinV = sa["nwin"], sb["nwin"]

    def node_w(Wn, bn):
        Wn, bn = Wn.astype(F32), bn.astype(F32)
        Wf = np.zeros((P, D), F16)
        Wf[64:80] = Wn[:16].astype(F16)
        Wf[80] = bn.astype(F16)
        Wh = np.ascontiguousarray(Wn[16:16 + D]).astype(F16)
        ctxproj = (ctx_emb.astype(F32) @ Wn[16 + D:16 + 2 * D]).astype(F16)
        We = np.ascontiguousarray(Wn[16 + 2 * D:16 + 3 * D]).astype(F16)
        return Wf, Wh, ctxproj, We

    WfC, WhC, ctxprojC, WeC = node_w(inp["W_c"], inp["b_c"])
    WfV, WhV, ctxprojV, WeV = node_w(inp["W_v"], inp["b_v"])

    common = dict(
        WembA=np.ascontiguousarray(W_vc[4:4 + D]).astype(F16),
        WsatA=np.vstack([W_vc[:4], b_vc[None, :]]).astype(F16),
        WembB=np.ascontiguousarray(W_cv[4:4 + D]).astype(F16),
        WsatB=np.vstack([W_cv[:4], b_cv[None, :]]).astype(F16),
        WfC=WfC, WhC=WhC, WeC=WeC, ctxprojC=ctxprojC,
        WfV=WfV, WhV=WhV, WeV=WeV, ctxprojV=ctxprojV,
    )

    in_maps = []
    for k in range(M):
        feC, emC, ohjC = _prep_nodes(
            inp["clause_feats"][k * cs:(k + 1) * cs],
            clause16[k * cs:(k + 1) * cs],
            clause_ctx[k * cs:(k + 1) * cs], cs, nwinC)
        feV, emV, ohjV = _prep_nodes(
            inp["var_feats"][k * vs:(k + 1) * vs],
            var16[k * vs:(k + 1) * vs],
            var_ctx[k * vs:(k + 1) * vs], vs, nwinV)
        in_maps.append(dict(
            gsA=pcA[k]["gs"], satpA=pcA[k]["satp"], dwA=pcA[k]["dw"],
            gsB=pcB[k]["gs"], satpB=pcB[k]["satp"], dwB=pcB[k]["dw"],
            feC=feC, emblC=emC, ohjC=ohjC,
            feV=feV, emblV=emV, ohjV=ohjV,
            **common,
        ))

    nc = _build(sa, sb, nwinC, nwinV)
    res = _run_spmd(nc, in_maps)

    new_clause = res["outC"].reshape(M, -1, D)[:, :cs].reshape(ncl, D)
    new_var = res["outV"].reshape(M, -1, D)[:, :vs].reshape(nv, D)
    acc = res["acc"].sum(0)  # [128 d, 128] cols 0:64 C, 64:128 V
    accC, accV = acc[:, :64], acc[:, 64:]

    cnt_cu = np.bincount(clause_ctx, minlength=nu).astype(F32)
    cnt_vu = np.bincount(var_ctx, minlength=nu).astype(F32)
    c_ctx = (accC / np.maximum(cnt_cu, 1.0)[None, :]).T
    v_ctx = (accV / np.maximum(cnt_vu, 1.0)[None, :]).T
    zu = np.concatenate([inp["ctx_feats"].astype(F32), c_ctx, v_ctx,
                         ctx_emb.astype(F32)], 1) @ inp["W_u"].astype(F32) \
        + inp["b_u"].astype(F32)
    new_ctx = np.where(zu >= 0, zu, 0.1 * zu).astype(F32)

    out = np.empty((ncl + nv + nu, D), F32)
    out[:ncl] = new_clause.astype(F32)
    out[ncl:ncl + nv] = new_var.astype(F32)
    out[ncl + nv:] = new_ctx
    return out


# revision 6
# speedup vs baseline: 1.3932x; 1.0919x over previous
"""Trainium2 Bass kernel for nn_MessageGNN (gnn_message_passing) — v2.

Destination-sharded edges across 8 cores (core k owns clauses
[k*50000,(k+1)*50000) and vars [k*12500,(k+1)*12500) plus every edge whose
destination falls in its slice), so segment sums are fully core-local.

One SPMD program (identical instruction stream on all 8 cores, per-core
data) dispatched with a single jit(shard_map) call — per-core schedule
constants (tiles per window, one-hot slice envelopes) are maxed/unioned
across cores on the host so the program is core-independent.

Per window of 1024 destinations:
  - src embeddings arrive as a host pre-gathered fp16 stream (edge-slot
    order), loaded feature-major with one HWDGE xbar-transpose DMA.
  - Edge MLP: per 128-edge tile, x^T / sat^T stationary against Wemb/Wsat,
    accumulating m[e,d] in PSUM; lrelu via ACT Prelu(alpha=.1) or DVE
    (0.1*x max x), alternating to balance engines.
  - Segment-mean via one-hot matmul: one-hot built by a two-scalar DVE op
    (iota == dst) * (1/cnt) at the per-tile envelope width, accumulated
    into a [128,1024] window PSUM as h^T.
  - Node MLP fused per 512-node half: feats+bias / ctx one-hot (host-built,
    DMA'd) / h / emb weight chunks; outputs transposed on PE (f16) and
    DMA'd out; phase-3 partial sums accumulate in a persistent PSUM tile.
The 64-row ctx update finishes on host from the per-core partial sums.
"""

import sys

sys.path.insert(0, "/opt/trn_rl_repo")

import numpy as np

NV, NC, NU, E, D = 100000, 400000, 64, 1200000, 128
M = 8
CS, VS = NC // M, NV // M
WIN = 1024
P = 128
PAD_DST = 1408.0

F16 = np.float16
F32 = np.float32


def _ceil(a, b):
    return -(-a // b)


def _prep_side(src, dst, sat, n_dst, tab16, we):
    """Edge-side prep: shared schedule + per-core slot tables.

    Returns (sched, percore) where sched is core-independent and percore[k]
    holds gs (pre-gathered src rows), satp ([5,S]), dw ([128,2*T_total]).
    """
    nwin = _ceil(n_dst, WIN)
    cores = []
    counts = np.zeros((M, nwin), np.int64)
    for k in range(M):
        base = k * n_dst
        mask = (dst >= base) & (dst < base + n_dst)
        es = np.nonzero(mask)[0]
        dstl = (dst[es] - base).astype(np.int64)
        order = np.argsort(dstl, kind="stable")
        es, dstl = es[order], dstl[order]
        counts[k] = np.bincount(dstl // WIN, minlength=nwin)
        cores.append((es, dstl))
    Tw = np.maximum(1, _ceil(counts.max(0), P)).astype(np.int64)
    toff = np.concatenate([[0], np.cumsum(Tw)])
    T_total = int(Tw.sum())
    S = T_total * P
    soff = toff * P

    slot_dst = np.full((M, S), -1, np.int64)
    slot_eid = np.full((M, S), -1, np.int64)
    for k, (es, dstl) in enumerate(cores):
        start = 0
        for wi in range(nwin):
            n = int(counts[k, wi])
            sl = slice(soff[wi], soff[wi] + n)
            slot_dst[k, sl] = dstl[start:start + n] - wi * WIN
            slot_eid[k, sl] = es[start:start + n]
            start += n

    # per-tile envelope (base/width across all cores) + 512-boundary slices
    tiles = []  # flat list over (window, tile): dict(base, width, slices)
    win_of_tile = np.repeat(np.arange(nwin), Tw)
    for ti in range(T_total):
        sl = slice(ti * P, (ti + 1) * P)
        dv = slot_dst[:, sl]
        real = dv >= 0
        if real.any():
            lo, hi = int(dv[real].min()), int(dv[real].max())
        else:
            lo, hi = 0, 0
        b0 = (lo // 32) * 32
        wd = _ceil(hi + 1 - b0, 32) * 32
        slices = []
        for h in (0, 1):
            s = max(b0, h * 512)
            e = min(b0 + wd, (h + 1) * 512)
            if s < e:
                slices.append((h, s - h * 512, s - b0, e - s))
        tiles.append(dict(base=b0, width=wd, slices=slices))

    # per-core tables
    percore = []
    base_of_slot = np.array([tiles[ti]["base"] for ti in range(T_total)],
                            np.int64).repeat(P)
    for k in range(M):
        dv, ev = slot_dst[k], slot_eid[k]
        real = dv >= 0
        dst_rel = np.full(S, PAD_DST, F32)
        dst_rel[real] = (dv[real] - base_of_slot[real]).astype(F32)
        wslot = np.zeros(S, F32)
        wslot[real] = we[dst[ev[real]]]
        dw = np.zeros((P, 2 * T_total), F32)
        dw[:, 0::2] = dst_rel.reshape(T_total, P).T
        dw[:, 1::2] = wslot.reshape(T_total, P).T
        satp = np.zeros((5, S), F16)
        satp[:4, real] = sat[ev[real]].T.astype(F16)
        satp[4, real] = 1.0
        gs = np.zeros((S, D), F16)
        gs[real] = tab16[src[ev[real]]]
        percore.append(dict(gs=gs, satp=satp, dw=dw))

    sched = dict(nwin=nwin, Tw=Tw.tolist(), toff=toff.tolist(),
                 soff=soff.tolist(), tiles=tiles, S=S, T_total=T_total,
                 Tmax=int(Tw.max()), Smax=int(Tw.max() * P))
    return sched, percore


def _prep_nodes(feats, emb16, ctx_ids, n_nodes, nwin):
    """Node tables for one core: fe [81, nwin*WIN], embl [128, nwin*WIN],
    ohj [128, nwin*512] (per-128-block ctx one-hot, u columns)."""
    Np = nwin * WIN
    fe = np.zeros((81, Np), F16)
    fe[ctx_ids, np.arange(n_nodes)] = 1.0          # rows 0:64 ctx one-hot
    fe[64:80, :n_nodes] = feats.T.astype(F16)      # rows 64:80 feats
    fe[80, :n_nodes] = 1.0                         # row 80 bias ones
    embl = np.zeros((P, Np), F16)
    embl[:, :n_nodes] = emb16.T
    nblk = nwin * 8
    ohj = np.zeros((P, nblk * 64), F16)
    node = np.arange(n_nodes)
    blk = node // P
    prow = node % P
    ohj[prow, blk * 64 + ctx_ids] = 1.0
    return fe, embl, ohj


def _build(sa, sb, nwinC, nwinV, repeat=1):
    import concourse.mybir as mybir
    import concourse.tile as tile
    from concourse import bacc
    from concourse.masks import make_identity

    f16, f32, i32 = mybir.dt.float16, mybir.dt.float32, mybir.dt.int32
    AF = mybir.ActivationFunctionType
    OP = mybir.AluOpType

    nc = bacc.Bacc("TRN2", target_bir_lowering=False, debug=False,
                   num_devices=1)
    io = {}

    def dram(name, shape, dt, kind="ExternalInput"):
        io[name] = nc.dram_tensor(name, list(shape), dt, kind=kind)
        return io[name]

    for side, s in (("A", sa), ("B", sb)):
        dram(f"gs{side}", [s["S"], D], f16)
        dram(f"satp{side}", [5, s["S"]], f16)
        dram(f"dw{side}", [P, 2 * s["T_total"]], f32)
        dram(f"Wemb{side}", [P, D], f16)
        dram(f"Wsat{side}", [5, D], f16)
    for sd, nwin in (("C", nwinC), ("V", nwinV)):
        dram(f"fe{sd}", [81, nwin * WIN], f16)
        dram(f"embl{sd}", [P, nwin * WIN], f16)
        dram(f"ohj{sd}", [P, nwin * 512], f16)
        dram(f"Wf{sd}", [P, D], f16)
        dram(f"Wh{sd}", [P, D], f16)
        dram(f"We{sd}", [P, D], f16)
        dram(f"ctxproj{sd}", [64, D], f16)
        dram(f"out{sd}", [nwin * 8, P, D], f16, kind="ExternalOutput")
    dram("acc", [P, P], f32, kind="ExternalOutput")

    with tile.TileContext(nc) as tc:
        with tc.tile_pool(name="const", bufs=1) as cpool, \
             tc.tile_pool(name="stage", bufs=2) as spool, \
             tc.tile_pool(name="meta", bufs=2) as mpool, \
             tc.tile_pool(name="work", bufs=3) as wpool, \
             tc.tile_pool(name="oh", bufs=10) as opool, \
             tc.tile_pool(name="node", bufs=2) as npool, \
             tc.tile_pool(name="psE", bufs=2, space="PSUM") as psE, \
             tc.tile_pool(name="psH", bufs=1, space="PSUM") as psH, \
             tc.tile_pool(name="psN", bufs=2, space="PSUM") as psN, \
             tc.tile_pool(name="psT", bufs=1, space="PSUM") as psT, \
             tc.tile_pool(name="psAcc", bufs=1, space="PSUM") as psA:

            identF = cpool.tile([P, P], f16)
            make_identity(nc, identF[:])
            iota_i = cpool.tile([P, WIN], i32)
            nc.gpsimd.iota(iota_i[:], pattern=[[1, WIN]], base=0,
                           channel_multiplier=0)
            iota16 = cpool.tile([P, WIN], f16)
            nc.vector.tensor_copy(iota16[:], iota_i[:])
            z128 = cpool.tile([1, P], f16)
            nc.gpsimd.memset(z128[:], 0.0)
            z512 = cpool.tile([1, 512], f16)
            nc.gpsimd.memset(z512[:], 0.0)

            wt = {}
            for nm in ("WembA", "WsatA", "WembB", "WsatB",
                       "WfC", "WhC", "WeC", "ctxprojC",
                       "WfV", "WhV", "WeV", "ctxprojV"):
                t = cpool.tile(list(io[nm].shape), f16, tag=nm)
                nc.sync.dma_start(t[:], io[nm][:])
                wt[nm] = t

            acc_ps = psA.tile([P, P], f32, name="accps")

            Smax = max(sa["Smax"], sb["Smax"])
            Tmax = max(sa["Tmax"], sb["Tmax"])

            def issue_edge(side, s, wi):
                T = s["Tw"][wi]
                t0, s0 = s["toff"][wi], s["soff"][wi]
                slots = T * P
                stage = spool.tile([P, Smax], f16, tag="stage")
                nc.sync.dma_start(stage[:, :slots],
                                  io[f"gs{side}"][s0:s0 + slots, :],
                                  transpose=True)
                satp = mpool.tile([5, Smax], f16, tag="satp")
                nc.scalar.dma_start(satp[:, :slots],
                                    io[f"satp{side}"][:, s0:s0 + slots])
                dw = mpool.tile([P, 2 * Tmax], f32, tag="dw")
                nc.scalar.dma_start(dw[:, :2 * T],
                                    io[f"dw{side}"][:, 2 * t0:2 * (t0 + T)])
                return dict(stage=stage, satp=satp, dw=dw, T=T, t0=t0)

            def issue_node(sd, wi):
                cga = wi * WIN
                fe = npool.tile([81, WIN], f16, tag="fe")
                nc.scalar.dma_start(fe[:], io[f"fe{sd}"][:, cga:cga + WIN])
                embl = npool.tile([P, WIN], f16, tag="embl")
                nc.sync.dma_start(embl[:], io[f"embl{sd}"][:, cga:cga + WIN])
                ohj = npool.tile([P, 512], f16, tag="ohj")
                nc.scalar.dma_start(ohj[:],
                                    io[f"ohj{sd}"][:, wi * 512:(wi + 1) * 512])
                return dict(fe=fe, embl=embl, ohj=ohj)

            jobs = [("A", "C", sa, wi) for wi in range(nwinC)] + \
                   [("B", "V", sb, wi) for wi in range(nwinV)]
            jobs = jobs * repeat
            rep_starts = set(range(0, len(jobs), len(jobs) // repeat))
            pf_e = issue_edge(jobs[0][0], jobs[0][2], jobs[0][3])
            pf_n = issue_node(jobs[0][1], jobs[0][3])
            for ji, (side, sd, s, wi) in enumerate(jobs):
                if ji in rep_starts:
                    nc.tensor.matmul(acc_ps[:], lhsT=z128[:], rhs=z512[:, :P],
                                     start=True, stop=False,
                                     skip_group_check=True)
                acccol = 0 if sd == "C" else 64
                cur_e, cur_n = pf_e, pf_n
                if ji + 1 < len(jobs):
                    nside, nsd, ns, nwi = jobs[ji + 1]
                    pf_e = issue_edge(nside, ns, nwi)
                    pf_n = issue_node(nsd, nwi)
                if True:
                    T, t0 = cur_e["T"], cur_e["t0"]
                    stage, satp, dw = (cur_e["stage"], cur_e["satp"],
                                       cur_e["dw"])
                    hps = [psH.tile([P, 512], f32, tag=f"h{i}", name=f"hps{i}")
                           for i in range(2)]
                    # last block touching each window half (for early hT copy)
                    bsplit = [0, 0]
                    for t in range(T):
                        for (h, _cw, _co, _ln) in s["tiles"][t0 + t]["slices"]:
                            bsplit[h] = max(bsplit[h], t // 4)
                    hT = npool.tile([P, WIN], f16, tag="hT")
                    def emit_oh(pend):
                        msb_p, b_p, nt_p, ohws_p = pend
                        for t4 in range(nt_p):
                            t = b_p * 4 + t4
                            tm = s["tiles"][t0 + t]
                            for (h, colw, colo, ln) in tm["slices"]:
                                nc.tensor.matmul(
                                    hps[h][:, colw:colw + ln],
                                    lhsT=msb_p[:, t4 * P:(t4 + 1) * P],
                                    rhs=ohws_p[t4][:, colo:colo + ln],
                                    start=False, stop=True,
                                    skip_group_check=True)

                    def emit_hcopy(b_done):
                        if b_done == bsplit[0]:
                            nc.vector.tensor_copy(hT[:, :512], hps[0][:])
                        if b_done == bsplit[1]:
                            nc.scalar.activation(hT[:, 512:], hps[1][:],
                                                 AF.Copy)

                    pend = None
                    for b in range(_ceil(T, 4)):
                        nt = min(4, T - b * 4)
                        mps = psE.tile([P, 512], f32, tag="mps")
                        for t4 in range(nt):
                            t = b * 4 + t4
                            nc.tensor.matmul(
                                mps[:, t4 * P:(t4 + 1) * P],
                                lhsT=stage[:, t * P:(t + 1) * P],
                                rhs=wt[f"Wemb{side}"][:],
                                start=True, stop=False)
                            nc.tensor.matmul(
                                mps[:, t4 * P:(t4 + 1) * P],
                                lhsT=satp[:, t * P:(t + 1) * P],
                                rhs=wt[f"Wsat{side}"][:],
                                start=False, stop=True)
                        if b == 0:
                            for i in range(2):
                                nc.tensor.matmul(hps[i][:], lhsT=z128[:],
                                                 rhs=z512[:], start=True,
                                                 stop=False,
                                                 skip_group_check=True)
                        ohws = []
                        for t4 in range(nt):
                            t = b * 4 + t4
                            tm = s["tiles"][t0 + t]
                            wd = tm["width"]
                            ohw = opool.tile([P, WIN], f16, tag="ohw")
                            nc.vector.tensor_scalar(
                                out=ohw[:, :wd], in0=iota16[:, :wd],
                                scalar1=dw[:, 2 * t:2 * t + 1],
                                scalar2=dw[:, 2 * t + 1:2 * t + 2],
                                op0=OP.is_equal, op1=OP.mult)
                            ohws.append(ohw)
                        msb = wpool.tile([P, 512], f16, tag="msb")
                        nc.scalar.activation(msb[:, :nt * P],
                                             mps[:, :nt * P],
                                             AF.Prelu, alpha=0.1)
                        if pend is not None:
                            emit_oh(pend)
                            emit_hcopy(pend[1])
                        pend = (msb, b, nt, ohws)
                    if pend is not None:
                        emit_oh(pend)
                        emit_hcopy(pend[1])

                    # ---- node phase for this window ----
                    fe, embl, ohj = cur_n["fe"], cur_n["embl"], cur_n["ohj"]
                    for g in (0, 1):
                        g0 = g * 512
                        nps = psN.tile([P, 512], f32, tag="nps")
                        nc.tensor.matmul(nps[:], lhsT=wt[f"Wf{sd}"][64:81, :],
                                         rhs=fe[64:81, g0:g0 + 512],
                                         start=True, stop=False)
                        nc.tensor.matmul(nps[:], lhsT=wt[f"ctxproj{sd}"][:],
                                         rhs=fe[:64, g0:g0 + 512],
                                         start=False, stop=False)
                        nc.tensor.matmul(nps[:], lhsT=wt[f"Wh{sd}"][:],
                                         rhs=hT[:, g0:g0 + 512],
                                         start=False, stop=False)
                        nc.tensor.matmul(nps[:], lhsT=wt[f"We{sd}"][:],
                                         rhs=embl[:, g0:g0 + 512],
                                         start=False, stop=True)
                        nsb = wpool.tile([P, 512], f16, tag="nsb")
                        nc.scalar.activation(nsb[:], nps[:], AF.Prelu,
                                             alpha=0.1)
                        tps = psT.tile([P, 512], f16, tag="tps")
                        for j in range(4):
                            nc.tensor.matmul(tps[:, j * P:(j + 1) * P],
                                             lhsT=nsb[:, j * P:(j + 1) * P],
                                             rhs=identF[:],
                                             is_transpose=True,
                                             skip_group_check=True)
                        osb = wpool.tile([P, 512], f16, tag="osb")
                        nc.vector.tensor_copy(osb[:], tps[:])
                        blk0 = (wi * 2 + g) * 4
                        eng = nc.sync if g == 0 else nc.scalar
                        eng.dma_start(
                            io[f"out{sd}"][blk0:blk0 + 4, :, :]
                            .rearrange("j p d -> p j d"),
                            osb[:].rearrange("p (j d) -> p j d", j=4))
                        for j in range(4):
                            nc.tensor.matmul(
                                acc_ps[:, acccol:acccol + 64],
                                lhsT=osb[:, j * P:(j + 1) * P],
                                rhs=ohj[:, (g * 4 + j) * 64:(g * 4 + j + 1) * 64],
                                start=False, stop=False,
                                skip_group_check=True)
            nc.tensor.matmul(acc_ps[:], lhsT=z128[:], rhs=z512[:, :P],
                             start=False, stop=True, skip_group_check=True)
            accsb = cpool.tile([P, P], f32, tag="accsb")
            nc.vector.tensor_copy(accsb[:], acc_ps[:])
            nc.sync.dma_start(io["acc"][:], accsb[:])
    nc.compile()
    return nc


_timing_handles = []


def _run_spmd(nc, in_maps):
    """One jit(shard_map) dispatch running the SPMD program on 8 cores."""
    import jax
    from jax.sharding import Mesh, PartitionSpec, NamedSharding
    from jax.experimental.shard_map import shard_map
    from concourse.bass2jax import _bass_exec_p, install_neuronx_cc_hook
    import concourse.mybir as mybir

    install_neuronx_cc_hook()
    devs = jax.devices()[:M]

    in_names, out_names, out_avals, zero_outs = [], [], [], []
    pid_name = None
    for alloc in nc.m.functions[0].allocations:
        if not isinstance(alloc, mybir.MemoryLocationSet):
            continue
        name = alloc.memorylocations[0].name
        if alloc.kind == "ExternalInput":
            if name == "partition_id":
                pid_name = name
            else:
                in_names.append(name)
        elif alloc.kind == "ExternalOutput":
            shape = tuple(alloc.tensor_shape)
            dtype = mybir.dt.np(alloc.dtype)
            out_names.append(name)
            out_avals.append(jax.core.ShapedArray(shape, dtype))
            zero_outs.append(np.zeros(shape, dtype))
    n_params = len(in_names)
    all_names = list(in_names) + list(out_names)
    if pid_name:
        all_names.append(pid_name)

    def _body(*args, _oa=tuple(out_avals), _an=tuple(all_names),
              _on=tuple(out_names), _nc=nc):
        return tuple(_bass_exec_p.bind(
            *args, out_avals=_oa, in_names=_an, out_names=_on,
            lowering_input_output_aliases=(),
            sim_require_finite=True, sim_require_nnan=True, nc=_nc,
        ))

    mesh = Mesh(np.asarray(devs), ("core",))
    nops = n_params + len(zero_outs) + (1 if pid_name else 0)
    spec = PartitionSpec("core")
    fn = jax.jit(shard_map(_body, mesh=mesh, in_specs=(spec,) * nops,
                           out_specs=(spec,) * len(out_names),
                           check_rep=False), keep_unused=True)

    concat_in = [np.concatenate([np.asarray(m[nm]) for m in in_maps], axis=0)
                 for nm in in_names]
    concat_zeros = [np.zeros((M * z.shape[0], *z.shape[1:]), z.dtype)
                    for z in zero_outs]
    args = concat_in + concat_zeros
    if pid_name:
        args.append(np.arange(M, dtype=np.uint32).reshape(M, 1))

    def _body_for(nc_alt):
        def _body2(*args):
            return tuple(_bass_exec_p.bind(
                *args, out_avals=tuple(out_avals), in_names=tuple(all_names),
                out_names=tuple(out_names),
                lowering_input_output_aliases=(),
                sim_require_finite=True, sim_require_nnan=True, nc=nc_alt,
            ))
        return jax.jit(shard_map(_body2, mesh=mesh, in_specs=(spec,) * nops,
                                 out_specs=(spec,) * len(out_names),
                                 check_rep=False), keep_unused=True)

    def make_loop_fn(K):
        """jit that executes the kernel K times back-to-back on-device
        (scan carries the output buffers), for dispatch-free timing."""
        n_outs = len(out_names)

        def _body_k(*args):
            data = args[:n_params]
            zeros = tuple(args[n_params:n_params + n_outs])
            pid = args[-1] if pid_name else None

            def step(carry, _):
                operands = list(data) + list(carry)
                if pid is not None:
                    operands.append(pid)
                outs = _bass_exec_p.bind(
                    *operands, out_avals=tuple(out_avals),
                    in_names=tuple(all_names), out_names=tuple(out_names),
                    lowering_input_output_aliases=(),
                    sim_require_finite=True, sim_require_nnan=True, nc=nc)
                return tuple(outs), None

            carry, _ = jax.lax.scan(step, zeros, None, length=K)
            return carry

        return jax.jit(shard_map(_body_k, mesh=mesh, in_specs=(spec,) * nops,
                                 out_specs=(spec,) * len(out_names),
                                 check_rep=False), keep_unused=True)

    out_arrs = fn(*args)
    res = [np.asarray(o) for o in out_arrs]
    _timing_handles.append(dict(fn=fn, args=args, devs=devs, mesh=mesh,
                                make_loop_fn=make_loop_fn,
                                make_body=_body_for))
    return {nm: res[i].reshape(M, *out_avals[i].shape)
            for i, nm in enumerate(out_names)}


def kernel(**inputs):
    inp = {k: np.asarray(v) for k, v in inputs.items()}
    var_emb, clause_emb, ctx_emb = (inp["var_emb"], inp["clause_emb"],
                                    inp["ctx_emb"])
    nv, ncl, nu = var_emb.shape[0], clause_emb.shape[0], ctx_emb.shape[0]
    cs, vs = ncl // M, nv // M

    a_src = inp["assigns_src"].astype(np.int64)
    a_dst = inp["assigns_dst"].astype(np.int64)
    c_src = inp["contains_src"].astype(np.int64)
    c_dst = inp["contains_dst"].astype(np.int64)
    var_ctx = inp["var_ctx"].astype(np.int64)
    clause_ctx = inp["clause_ctx"].astype(np.int64)

    cnt_c = np.bincount(a_dst, minlength=ncl).astype(F32)
    cnt_v = np.bincount(c_dst, minlength=nv).astype(F32)
    we_c = 1.0 / np.maximum(cnt_c, 1.0)
    we_v = 1.0 / np.maximum(cnt_v, 1.0)

    var16 = var_emb.astype(F16)
    clause16 = clause_emb.astype(F16)

    W_vc, b_vc = inp["W_vc"].astype(F32), inp["b_vc"].astype(F32)
    W_cv, b_cv = inp["W_cv"].astype(F32), inp["b_cv"].astype(F32)

    sa, pcA = _prep_side(a_src, a_dst, inp["edge_sat_vc"], cs, var16, we_c)
    sb, pcB = _prep_side(c_src, c_dst, inp["edge_sat_cv"], vs, clause16, we_v)
    nwinC, nwinV = sa["nwin"], sb["nwin"]

    def node_w(Wn, bn):
        Wn, bn = Wn.astype(F32), bn.astype(F32)
        Wf = np.zeros((P, D), F16)
        Wf[64:80] = Wn[:16].astype(F16)
        Wf[80] = bn.astype(F16)
        Wh = np.ascontiguousarray(Wn[16:16 + D]).astype(F16)
        ctxproj = (ctx_emb.astype(F32) @ Wn[16 + D:16 + 2 * D]).astype(F16)
        We = np.ascontiguousarray(Wn[16 + 2 * D:16 + 3 * D]).astype(F16)
        return Wf, Wh, ctxproj, We

    WfC, WhC, ctxprojC, WeC = node_w(inp["W_c"], inp["b_c"])
    WfV, WhV, ctxprojV, WeV = node_w(inp["W_v"], inp["b_v"])

    common = dict(
        WembA=np.ascontiguousarray(W_vc[4:4 + D]).astype(F16),
        WsatA=np.vstack([W_vc[:4], b_vc[None, :]]).astype(F16),
        WembB=np.ascontiguousarray(W_cv[4:4 + D]).astype(F16),
        WsatB=np.vstack([W_cv[:4], b_cv[None, :]]).astype(F16),
        WfC=WfC, WhC=WhC, WeC=WeC, ctxprojC=ctxprojC,
        WfV=WfV, WhV=WhV, WeV=WeV, ctxprojV=ctxprojV,
    )

    in_maps = []
    for k in range(M):
        feC, emC, ohjC = _prep_nodes(
            inp["clause_feats"][k * cs:(k + 1) * cs],
            clause16[k * cs:(k + 1) * cs],
            clause_ctx[k * cs:(k + 1) * cs], cs, nwinC)
        feV, emV, ohjV = _prep_nodes(
            inp["var_feats"][k * vs:(k + 1) * vs],
            var16[k * vs:(k + 1) * vs],
            var_ctx[k * vs:(k + 1) * vs], vs, nwinV)
        in_maps.append(dict(
            gsA=pcA[k]["gs"], satpA=pcA[k]["satp"], dwA=pcA[k]["dw"],
            gsB=pcB[k]["gs"], satpB=pcB[k]["satp"], dwB=pcB[k]["dw"],
            feC=feC, emblC=emC, ohjC=ohjC,
            feV=feV, emblV=emV, ohjV=ohjV,
            **common,
        ))

    nc = _build(sa, sb, nwinC, nwinV)
    res = _run_spmd(nc, in_maps)
    _timing_handles[-1]["make_repeat"] = (
        lambda r: _timing_handles[-1]["make_body"](
            _build(sa, sb, nwinC, nwinV, repeat=r)))

    new_clause = res["outC"].reshape(M, -1, D)[:, :cs].reshape(ncl, D)
    new_var = res["outV"].reshape(M, -1, D)[:, :vs].reshape(nv, D)
    acc = res["acc"].sum(0)  # [128 d, 128] cols 0:64 C, 64:128 V
    accC, accV = acc[:, :64], acc[:, 64:]

    cnt_cu = np.bincount(clause_ctx, minlength=nu).astype(F32)
    cnt_vu = np.bincount(var_ctx, minlength=nu).astype(F32)
    c_ctx = (accC / np.maximum(cnt_cu, 1.0)[None, :]).T
    v_ctx = (accV / np.maximum(cnt_vu, 1.0)[None, :]).T
    zu = np.concatenate([inp["ctx_feats"].astype(F32), c_ctx, v_ctx,
                         ctx_emb.astype(F32)], 1) @ inp["W_u"].astype(F32) \
        + inp["b_u"].astype(F32)
    new_ctx = np.where(zu >= 0, zu, 0.1 * zu).astype(F32)

    out = np.empty((ncl + nv + nu, D), F32)
    out[:ncl] = new_clause.astype(F32)
    out[ncl:ncl + nv] = new_var.astype(F32)
    out[ncl + nv:] = new_ctx
    return out


# revision 7
# speedup vs baseline: 1.5936x; 1.1439x over previous
"""Trainium2 Bass kernel for nn_MessageGNN (gnn_message_passing) — v2.

Destination-sharded edges across 8 cores (core k owns clauses
[k*50000,(k+1)*50000) and vars [k*12500,(k+1)*12500) plus every edge whose
destination falls in its slice), so segment sums are fully core-local.

One SPMD program (identical instruction stream on all 8 cores, per-core
data) dispatched with a single jit(shard_map) call — per-core schedule
constants (tiles per window, one-hot slice envelopes) are maxed/unioned
across cores on the host so the program is core-independent.

Per window of 1024 destinations:
  - src embeddings arrive as a host pre-gathered fp16 stream (edge-slot
    order), loaded feature-major with one HWDGE xbar-transpose DMA.
  - Edge MLP: per 128-edge tile, x^T / sat^T stationary against Wemb/Wsat,
    accumulating m[e,d] in PSUM; lrelu via ACT Prelu(alpha=.1) or DVE
    (0.1*x max x), alternating to balance engines.
  - Segment-mean via one-hot matmul: one-hot built by a two-scalar DVE op
    (iota == dst) * (1/cnt) at the per-tile envelope width, accumulated
    into a [128,1024] window PSUM as h^T.
  - Node MLP fused per 512-node half: feats+bias / ctx one-hot (host-built,
    DMA'd) / h / emb weight chunks; outputs transposed on PE (f16) and
    DMA'd out; phase-3 partial sums accumulate in a persistent PSUM tile.
The 64-row ctx update finishes on host from the per-core partial sums.
"""

import sys

sys.path.insert(0, "/opt/trn_rl_repo")

import numpy as np

NV, NC, NU, E, D = 100000, 400000, 64, 1200000, 128
M = 8
CS, VS = NC // M, NV // M
WIN = 1024
P = 128
PAD_DST = 1408.0

F16 = np.float16
F32 = np.float32


def _ceil(a, b):
    return -(-a // b)


def _prep_side(src, dst, sat, n_dst, tab16, we):
    """Edge-side prep: shared schedule + per-core slot tables.

    Returns (sched, percore) where sched is core-independent and percore[k]
    holds gs (pre-gathered src rows), satp ([5,S]), dw ([128,2*T_total]).
    """
    nwin = _ceil(n_dst, WIN)
    cores = []
    counts = np.zeros((M, nwin), np.int64)
    for k in range(M):
        base = k * n_dst
        mask = (dst >= base) & (dst < base + n_dst)
        es = np.nonzero(mask)[0]
        dstl = (dst[es] - base).astype(np.int64)
        order = np.argsort(dstl, kind="stable")
        es, dstl = es[order], dstl[order]
        counts[k] = np.bincount(dstl // WIN, minlength=nwin)
        cores.append((es, dstl))
    Tw = np.maximum(1, _ceil(counts.max(0), P)).astype(np.int64)
    toff = np.concatenate([[0], np.cumsum(Tw)])
    T_total = int(Tw.sum())
    S = T_total * P
    soff = toff * P

    slot_dst = np.full((M, S), -1, np.int64)
    slot_eid = np.full((M, S), -1, np.int64)
    for k, (es, dstl) in enumerate(cores):
        start = 0
        for wi in range(nwin):
            n = int(counts[k, wi])
            sl = slice(soff[wi], soff[wi] + n)
            slot_dst[k, sl] = dstl[start:start + n] - wi * WIN
            slot_eid[k, sl] = es[start:start + n]
            start += n

    # per-tile envelope (base/width across all cores) + 512-boundary slices
    tiles = []  # flat list over (window, tile): dict(base, width, slices)
    win_of_tile = np.repeat(np.arange(nwin), Tw)
    for ti in range(T_total):
        sl = slice(ti * P, (ti + 1) * P)
        dv = slot_dst[:, sl]
        real = dv >= 0
        if real.any():
            lo, hi = int(dv[real].min()), int(dv[real].max())
        else:
            lo, hi = 0, 0
        b0 = (lo // 32) * 32
        wd = _ceil(hi + 1 - b0, 32) * 32
        slices = []
        for h in (0, 1):
            s = max(b0, h * 512)
            e = min(b0 + wd, (h + 1) * 512)
            if s < e:
                slices.append((h, s - h * 512, s - b0, e - s))
        tiles.append(dict(base=b0, width=wd, slices=slices))

    # per-core tables
    percore = []
    base_of_slot = np.array([tiles[ti]["base"] for ti in range(T_total)],
                            np.int64).repeat(P)
    for k in range(M):
        dv, ev = slot_dst[k], slot_eid[k]
        real = dv >= 0
        dst_rel = np.full(S, PAD_DST, F32)
        dst_rel[real] = (dv[real] - base_of_slot[real]).astype(F32)
        wslot = np.zeros(S, F32)
        wslot[real] = we[dst[ev[real]]]
        dw = np.zeros((P, 2 * T_total), F32)
        dw[:, 0::2] = dst_rel.reshape(T_total, P).T
        dw[:, 1::2] = wslot.reshape(T_total, P).T
        satp = np.zeros((5, S), F16)
        satp[:4, real] = sat[ev[real]].T.astype(F16)
        satp[4, real] = 1.0
        gs = np.zeros((S, D), F16)
        gs[real] = tab16[src[ev[real]]]
        percore.append(dict(gs=gs, satp=satp, dw=dw))

    sched = dict(nwin=nwin, Tw=Tw.tolist(), toff=toff.tolist(),
                 soff=soff.tolist(), tiles=tiles, S=S, T_total=T_total,
                 Tmax=int(Tw.max()), Smax=int(Tw.max() * P))
    return sched, percore


def _prep_nodes(feats, emb16, ctx_ids, n_nodes, nwin):
    """Node tables for one core: fe [81, nwin*WIN], embl [128, nwin*WIN],
    ohj [128, nwin*512] (per-128-block ctx one-hot, u columns)."""
    Np = nwin * WIN
    fe = np.zeros((81, Np), F16)
    fe[ctx_ids, np.arange(n_nodes)] = 1.0          # rows 0:64 ctx one-hot
    fe[64:80, :n_nodes] = feats.T.astype(F16)      # rows 64:80 feats
    fe[80, :n_nodes] = 1.0                         # row 80 bias ones
    embl = np.zeros((P, Np), F16)
    embl[:, :n_nodes] = emb16.T
    nblk = nwin * 8
    ohj = np.zeros((P, nblk * 64), F16)
    node = np.arange(n_nodes)
    blk = node // P
    prow = node % P
    ohj[prow, blk * 64 + ctx_ids] = 1.0
    return fe, embl, ohj


def _build(sa, sb, nwinC, nwinV, repeat=1):
    import concourse.mybir as mybir
    import concourse.tile as tile
    from concourse import bacc
    from concourse.masks import make_identity

    f16, f32, i32 = mybir.dt.float16, mybir.dt.float32, mybir.dt.int32
    AF = mybir.ActivationFunctionType
    OP = mybir.AluOpType

    nc = bacc.Bacc("TRN2", target_bir_lowering=False, debug=False,
                   num_devices=1)
    io = {}

    def dram(name, shape, dt, kind="ExternalInput"):
        io[name] = nc.dram_tensor(name, list(shape), dt, kind=kind)
        return io[name]

    for side, s in (("A", sa), ("B", sb)):
        dram(f"gs{side}", [s["S"], D], f16)
        dram(f"satp{side}", [5, s["S"]], f16)
        dram(f"dw{side}", [P, 2 * s["T_total"]], f32)
        dram(f"Wemb{side}", [P, D], f16)
        dram(f"Wsat{side}", [5, D], f16)
    for sd, nwin in (("C", nwinC), ("V", nwinV)):
        dram(f"fe{sd}", [81, nwin * WIN], f16)
        dram(f"embl{sd}", [P, nwin * WIN], f16)
        dram(f"ohj{sd}", [P, nwin * 512], f16)
        dram(f"Wf{sd}", [P, D], f16)
        dram(f"Wh{sd}", [P, D], f16)
        dram(f"We{sd}", [P, D], f16)
        dram(f"ctxproj{sd}", [64, D], f16)
        dram(f"out{sd}", [nwin * 8, P, D], f16, kind="ExternalOutput")
    dram("acc", [P, P], f32, kind="ExternalOutput")

    with tile.TileContext(nc) as tc:
        with tc.tile_pool(name="const", bufs=1) as cpool, \
             tc.tile_pool(name="stage", bufs=2) as spool, \
             tc.tile_pool(name="meta", bufs=2) as mpool, \
             tc.tile_pool(name="work", bufs=3) as wpool, \
             tc.tile_pool(name="oh", bufs=10) as opool, \
             tc.tile_pool(name="node", bufs=2) as npool, \
             tc.tile_pool(name="psE", bufs=3, space="PSUM") as psE, \
             tc.tile_pool(name="psH", bufs=1, space="PSUM") as psH, \
             tc.tile_pool(name="psN", bufs=1, space="PSUM") as psN, \
             tc.tile_pool(name="psT", bufs=1, space="PSUM") as psT:

            identF = cpool.tile([P, P], f16)
            make_identity(nc, identF[:])
            iota_i = cpool.tile([P, WIN], i32)
            nc.gpsimd.iota(iota_i[:], pattern=[[1, WIN]], base=0,
                           channel_multiplier=0)
            iota16 = cpool.tile([P, WIN], f16)
            nc.vector.tensor_copy(iota16[:], iota_i[:])
            z128 = cpool.tile([1, P], f16)
            nc.gpsimd.memset(z128[:], 0.0)
            z512 = cpool.tile([1, 512], f16)
            nc.gpsimd.memset(z512[:], 0.0)

            wt = {}
            for nm in ("WembA", "WsatA", "WembB", "WsatB",
                       "WfC", "WhC", "WeC", "ctxprojC",
                       "WfV", "WhV", "WeV", "ctxprojV"):
                t = cpool.tile(list(io[nm].shape), f16, tag=nm)
                nc.sync.dma_start(t[:], io[nm][:])
                wt[nm] = t

            acc_sb = cpool.tile([P, P], f32, tag="accsbuf")

            Smax = max(sa["Smax"], sb["Smax"])
            Tmax = max(sa["Tmax"], sb["Tmax"])

            def issue_edge(side, s, wi):
                T = s["Tw"][wi]
                t0, s0 = s["toff"][wi], s["soff"][wi]
                slots = T * P
                stage = spool.tile([P, Smax], f16, tag="stage")
                nc.sync.dma_start(stage[:, :slots],
                                  io[f"gs{side}"][s0:s0 + slots, :],
                                  transpose=True)
                satp = mpool.tile([5, Smax], f16, tag="satp")
                nc.sync.dma_start(satp[:, :slots],
                                    io[f"satp{side}"][:, s0:s0 + slots])
                dw = mpool.tile([P, 2 * Tmax], f32, tag="dw")
                nc.sync.dma_start(dw[:, :2 * T],
                                    io[f"dw{side}"][:, 2 * t0:2 * (t0 + T)])
                return dict(stage=stage, satp=satp, dw=dw, T=T, t0=t0)

            def issue_node(sd, wi):
                cga = wi * WIN
                fe = npool.tile([81, WIN], f16, tag="fe")
                nc.sync.dma_start(fe[:], io[f"fe{sd}"][:, cga:cga + WIN])
                embl = npool.tile([P, WIN], f16, tag="embl")
                nc.sync.dma_start(embl[:], io[f"embl{sd}"][:, cga:cga + WIN])
                ohj = npool.tile([P, 512], f16, tag="ohj")
                nc.sync.dma_start(ohj[:],
                                    io[f"ohj{sd}"][:, wi * 512:(wi + 1) * 512])
                return dict(fe=fe, embl=embl, ohj=ohj)

            jobs = [("A", "C", sa, wi) for wi in range(nwinC)] + \
                   [("B", "V", sb, wi) for wi in range(nwinV)]
            jobs = jobs * repeat
            rep_starts = set(range(0, len(jobs), len(jobs) // repeat))
            pf_e = issue_edge(jobs[0][0], jobs[0][2], jobs[0][3])
            pf_n = issue_node(jobs[0][1], jobs[0][3])
            for ji, (side, sd, s, wi) in enumerate(jobs):
                if ji in rep_starts:
                    nc.vector.memset(acc_sb[:], 0.0)
                acccol = 0 if sd == "C" else 64
                cur_e, cur_n = pf_e, pf_n
                if ji + 1 < len(jobs):
                    nside, nsd, ns, nwi = jobs[ji + 1]
                    pf_e = issue_edge(nside, ns, nwi)
                    pf_n = issue_node(nsd, nwi)
                if True:
                    T, t0 = cur_e["T"], cur_e["t0"]
                    stage, satp, dw = (cur_e["stage"], cur_e["satp"],
                                       cur_e["dw"])
                    hps = [psH.tile([P, 512], f32, tag=f"h{i}", name=f"hps{i}")
                           for i in range(2)]
                    # last block touching each window half (for early hT copy)
                    bsplit = [0, 0]
                    for t in range(T):
                        for (h, _cw, _co, _ln) in s["tiles"][t0 + t]["slices"]:
                            bsplit[h] = max(bsplit[h], t // 4)
                    hT = npool.tile([P, WIN], f16, tag="hT")
                    def emit_oh(pend):
                        msb_p, b_p, nt_p, ohws_p = pend
                        for t4 in range(nt_p):
                            t = b_p * 4 + t4
                            tm = s["tiles"][t0 + t]
                            for (h, colw, colo, ln) in tm["slices"]:
                                nc.tensor.matmul(
                                    hps[h][:, colw:colw + ln],
                                    lhsT=msb_p[:, t4 * P:(t4 + 1) * P],
                                    rhs=ohws_p[t4][:, colo:colo + ln],
                                    start=False, stop=True,
                                    skip_group_check=True)

                    def emit_hcopy(b_done):
                        if b_done == bsplit[0]:
                            nc.vector.tensor_copy(hT[:, :512], hps[0][:])
                        if b_done == bsplit[1]:
                            nc.scalar.activation(hT[:, 512:], hps[1][:],
                                                 AF.Copy)

                    pend = None
                    for b in range(_ceil(T, 4)):
                        nt = min(4, T - b * 4)
                        mps = psE.tile([P, 512], f32, tag="mps")
                        for t4 in range(nt):
                            t = b * 4 + t4
                            nc.tensor.matmul(
                                mps[:, t4 * P:(t4 + 1) * P],
                                lhsT=stage[:, t * P:(t + 1) * P],
                                rhs=wt[f"Wemb{side}"][:],
                                start=True, stop=False)
                            nc.tensor.matmul(
                                mps[:, t4 * P:(t4 + 1) * P],
                                lhsT=satp[:, t * P:(t + 1) * P],
                                rhs=wt[f"Wsat{side}"][:],
                                start=False, stop=True)
                        if b == 0:
                            for i in range(2):
                                nc.vector.memset(hps[i][:], 0.0)
                        ohws = []
                        for t4 in range(nt):
                            t = b * 4 + t4
                            tm = s["tiles"][t0 + t]
                            wd = tm["width"]
                            ohw = opool.tile([P, WIN], f16, tag="ohw")
                            nc.vector.tensor_scalar(
                                out=ohw[:, :wd], in0=iota16[:, :wd],
                                scalar1=dw[:, 2 * t:2 * t + 1],
                                scalar2=dw[:, 2 * t + 1:2 * t + 2],
                                op0=OP.is_equal, op1=OP.mult)
                            ohws.append(ohw)
                        msb = wpool.tile([P, 512], f16, tag="msb")
                        nc.scalar.activation(msb[:, :nt * P],
                                             mps[:, :nt * P],
                                             AF.Prelu, alpha=0.1)
                        if pend is not None:
                            emit_oh(pend)
                            emit_hcopy(pend[1])
                        pend = (msb, b, nt, ohws)
                    if pend is not None:
                        emit_oh(pend)
                        emit_hcopy(pend[1])

                    # ---- node phase for this window ----
                    fe, embl, ohj = cur_n["fe"], cur_n["embl"], cur_n["ohj"]
                    for g in (0, 1):
                        g0 = g * 512
                        nps = psN.tile([P, 512], f32, tag="nps")
                        nc.tensor.matmul(nps[:], lhsT=wt[f"Wf{sd}"][64:81, :],
                                         rhs=fe[64:81, g0:g0 + 512],
                                         start=True, stop=False)
                        nc.tensor.matmul(nps[:], lhsT=wt[f"ctxproj{sd}"][:],
                                         rhs=fe[:64, g0:g0 + 512],
                                         start=False, stop=False)
                        nc.tensor.matmul(nps[:], lhsT=wt[f"Wh{sd}"][:],
                                         rhs=hT[:, g0:g0 + 512],
                                         start=False, stop=False)
                        nc.tensor.matmul(nps[:], lhsT=wt[f"We{sd}"][:],
                                         rhs=embl[:, g0:g0 + 512],
                                         start=False, stop=True)
                        nsb = wpool.tile([P, 512], f16, tag="nsb")
                        nc.scalar.activation(nsb[:], nps[:], AF.Prelu,
                                             alpha=0.1)
                        tps = psT.tile([P, 512], f16, tag="tps")
                        for j in range(4):
                            nc.tensor.matmul(tps[:, j * P:(j + 1) * P],
                                             lhsT=nsb[:, j * P:(j + 1) * P],
                                             rhs=identF[:],
                                             is_transpose=True,
                                             skip_group_check=True)
                        osb = wpool.tile([P, 512], f16, tag="osb")
                        nc.vector.tensor_copy(osb[:], tps[:])
                        blk0 = (wi * 2 + g) * 4
                        nc.sync.dma_start(
                            io[f"out{sd}"][blk0:blk0 + 4, :, :]
                            .rearrange("j p d -> p j d"),
                            osb[:].rearrange("p (j d) -> p j d", j=4))
                        aps = psT.tile([P, 64], f32, tag="aps")
                        for j in range(4):
                            nc.tensor.matmul(
                                aps[:],
                                lhsT=osb[:, j * P:(j + 1) * P],
                                rhs=ohj[:, (g * 4 + j) * 64:(g * 4 + j + 1) * 64],
                                start=(j == 0), stop=(j == 3),
                                skip_group_check=True)
                        nc.vector.tensor_add(acc_sb[:, acccol:acccol + 64],
                                             acc_sb[:, acccol:acccol + 64],
                                             aps[:])
            nc.sync.dma_start(io["acc"][:], acc_sb[:])
    nc.compile()
    return nc


_timing_handles = []


def _run_spmd(nc, in_maps):
    """One jit(shard_map) dispatch running the SPMD program on 8 cores."""
    import jax
    from jax.sharding import Mesh, PartitionSpec, NamedSharding
    from jax.experimental.shard_map import shard_map
    from concourse.bass2jax import _bass_exec_p, install_neuronx_cc_hook
    import concourse.mybir as mybir

    install_neuronx_cc_hook()
    devs = jax.devices()[:M]

    in_names, out_names, out_avals, zero_outs = [], [], [], []
    pid_name = None
    for alloc in nc.m.functions[0].allocations:
        if not isinstance(alloc, mybir.MemoryLocationSet):
            continue
        name = alloc.memorylocations[0].name
        if alloc.kind == "ExternalInput":
            if name == "partition_id":
                pid_name = name
            else:
                in_names.append(name)
        elif alloc.kind == "ExternalOutput":
            shape = tuple(alloc.tensor_shape)
            dtype = mybir.dt.np(alloc.dtype)
            out_names.append(name)
            out_avals.append(jax.core.ShapedArray(shape, dtype))
            zero_outs.append(np.zeros(shape, dtype))
    n_params = len(in_names)
    all_names = list(in_names) + list(out_names)
    if pid_name:
        all_names.append(pid_name)

    def _body(*args, _oa=tuple(out_avals), _an=tuple(all_names),
              _on=tuple(out_names), _nc=nc):
        return tuple(_bass_exec_p.bind(
            *args, out_avals=_oa, in_names=_an, out_names=_on,
            lowering_input_output_aliases=(),
            sim_require_finite=True, sim_require_nnan=True, nc=_nc,
        ))

    mesh = Mesh(np.asarray(devs), ("core",))
    nops = n_params + len(zero_outs) + (1 if pid_name else 0)
    spec = PartitionSpec("core")
    fn = jax.jit(shard_map(_body, mesh=mesh, in_specs=(spec,) * nops,
                           out_specs=(spec,) * len(out_names),
                           check_rep=False), keep_unused=True)

    concat_in = [np.concatenate([np.asarray(m[nm]) for m in in_maps], axis=0)
                 for nm in in_names]
    concat_zeros = [np.zeros((M * z.shape[0], *z.shape[1:]), z.dtype)
                    for z in zero_outs]
    args = concat_in + concat_zeros
    if pid_name:
        args.append(np.arange(M, dtype=np.uint32).reshape(M, 1))

    def _body_for(nc_alt):
        def _body2(*args):
            return tuple(_bass_exec_p.bind(
                *args, out_avals=tuple(out_avals), in_names=tuple(all_names),
                out_names=tuple(out_names),
                lowering_input_output_aliases=(),
                sim_require_finite=True, sim_require_nnan=True, nc=nc_alt,
            ))
        return jax.jit(shard_map(_body2, mesh=mesh, in_specs=(spec,) * nops,
                                 out_specs=(spec,) * len(out_names),
                                 check_rep=False), keep_unused=True)

    def make_loop_fn(K):
        """jit that executes the kernel K times back-to-back on-device
        (scan carries the output buffers), for dispatch-free timing."""
        n_outs = len(out_names)

        def _body_k(*args):
            data = args[:n_params]
            zeros = tuple(args[n_params:n_params + n_outs])
            pid = args[-1] if pid_name else None

            def step(carry, _):
                operands = list(data) + list(carry)
                if pid is not None:
                    operands.append(pid)
                outs = _bass_exec_p.bind(
                    *operands, out_avals=tuple(out_avals),
                    in_names=tuple(all_names), out_names=tuple(out_names),
                    lowering_input_output_aliases=(),
                    sim_require_finite=True, sim_require_nnan=True, nc=nc)
                return tuple(outs), None

            carry, _ = jax.lax.scan(step, zeros, None, length=K)
            return carry

        return jax.jit(shard_map(_body_k, mesh=mesh, in_specs=(spec,) * nops,
                                 out_specs=(spec,) * len(out_names),
                                 check_rep=False), keep_unused=True)

    out_arrs = fn(*args)
    res = [np.asarray(o) for o in out_arrs]
    _timing_handles.append(dict(fn=fn, args=args, devs=devs, mesh=mesh,
                                make_loop_fn=make_loop_fn,
                                make_body=_body_for))
    return {nm: res[i].reshape(M, *out_avals[i].shape)
            for i, nm in enumerate(out_names)}


def kernel(**inputs):
    inp = {k: np.asarray(v) for k, v in inputs.items()}
    var_emb, clause_emb, ctx_emb = (inp["var_emb"], inp["clause_emb"],
                                    inp["ctx_emb"])
    nv, ncl, nu = var_emb.shape[0], clause_emb.shape[0], ctx_emb.shape[0]
    cs, vs = ncl // M, nv // M

    a_src = inp["assigns_src"].astype(np.int64)
    a_dst = inp["assigns_dst"].astype(np.int64)
    c_src = inp["contains_src"].astype(np.int64)
    c_dst = inp["contains_dst"].astype(np.int64)
    var_ctx = inp["var_ctx"].astype(np.int64)
    clause_ctx = inp["clause_ctx"].astype(np.int64)

    cnt_c = np.bincount(a_dst, minlength=ncl).astype(F32)
    cnt_v = np.bincount(c_dst, minlength=nv).astype(F32)
    we_c = 1.0 / np.maximum(cnt_c, 1.0)
    we_v = 1.0 / np.maximum(cnt_v, 1.0)

    var16 = var_emb.astype(F16)
    clause16 = clause_emb.astype(F16)

    W_vc, b_vc = inp["W_vc"].astype(F32), inp["b_vc"].astype(F32)
    W_cv, b_cv = inp["W_cv"].astype(F32), inp["b_cv"].astype(F32)

    sa, pcA = _prep_side(a_src, a_dst, inp["edge_sat_vc"], cs, var16, we_c)
    sb, pcB = _prep_side(c_src, c_dst, inp["edge_sat_cv"], vs, clause16, we_v)
    nwinC, nwinV = sa["nwin"], sb["nwin"]

    def node_w(Wn, bn):
        Wn, bn = Wn.astype(F32), bn.astype(F32)
        Wf = np.zeros((P, D), F16)
        Wf[64:80] = Wn[:16].astype(F16)
        Wf[80] = bn.astype(F16)
        Wh = np.ascontiguousarray(Wn[16:16 + D]).astype(F16)
        ctxproj = (ctx_emb.astype(F32) @ Wn[16 + D:16 + 2 * D]).astype(F16)
        We = np.ascontiguousarray(Wn[16 + 2 * D:16 + 3 * D]).astype(F16)
        return Wf, Wh, ctxproj, We

    WfC, WhC, ctxprojC, WeC = node_w(inp["W_c"], inp["b_c"])
    WfV, WhV, ctxprojV, WeV = node_w(inp["W_v"], inp["b_v"])

    common = dict(
        WembA=np.ascontiguousarray(W_vc[4:4 + D]).astype(F16),
        WsatA=np.vstack([W_vc[:4], b_vc[None, :]]).astype(F16),
        WembB=np.ascontiguousarray(W_cv[4:4 + D]).astype(F16),
        WsatB=np.vstack([W_cv[:4], b_cv[None, :]]).astype(F16),
        WfC=WfC, WhC=WhC, WeC=WeC, ctxprojC=ctxprojC,
        WfV=WfV, WhV=WhV, WeV=WeV, ctxprojV=ctxprojV,
    )

    in_maps = []
    for k in range(M):
        feC, emC, ohjC = _prep_nodes(
            inp["clause_feats"][k * cs:(k + 1) * cs],
            clause16[k * cs:(k + 1) * cs],
            clause_ctx[k * cs:(k + 1) * cs], cs, nwinC)
        feV, emV, ohjV = _prep_nodes(
            inp["var_feats"][k * vs:(k + 1) * vs],
            var16[k * vs:(k + 1) * vs],
            var_ctx[k * vs:(k + 1) * vs], vs, nwinV)
        in_maps.append(dict(
            gsA=pcA[k]["gs"], satpA=pcA[k]["satp"], dwA=pcA[k]["dw"],
            gsB=pcB[k]["gs"], satpB=pcB[k]["satp"], dwB=pcB[k]["dw"],
            feC=feC, emblC=emC, ohjC=ohjC,
            feV=feV, emblV=emV, ohjV=ohjV,
            **common,
        ))

    nc = _build(sa, sb, nwinC, nwinV)
    res = _run_spmd(nc, in_maps)
    _timing_handles[-1]["make_repeat"] = (
        lambda r: _timing_handles[-1]["make_body"](
            _build(sa, sb, nwinC, nwinV, repeat=r)))

    new_clause = res["outC"].reshape(M, -1, D)[:, :cs].reshape(ncl, D)
    new_var = res["outV"].reshape(M, -1, D)[:, :vs].reshape(nv, D)
    acc = res["acc"].sum(0)  # [128 d, 128] cols 0:64 C, 64:128 V
    accC, accV = acc[:, :64], acc[:, 64:]

    cnt_cu = np.bincount(clause_ctx, minlength=nu).astype(F32)
    cnt_vu = np.bincount(var_ctx, minlength=nu).astype(F32)
    c_ctx = (accC / np.maximum(cnt_cu, 1.0)[None, :]).T
    v_ctx = (accV / np.maximum(cnt_vu, 1.0)[None, :]).T
    zu = np.concatenate([inp["ctx_feats"].astype(F32), c_ctx, v_ctx,
                         ctx_emb.astype(F32)], 1) @ inp["W_u"].astype(F32) \
        + inp["b_u"].astype(F32)
    new_ctx = np.where(zu >= 0, zu, 0.1 * zu).astype(F32)

    out = np.empty((ncl + nv + nu, D), F32)
    out[:ncl] = new_clause.astype(F32)
    out[ncl:ncl + nv] = new_var.astype(F32)
    out[ncl + nv:] = new_ctx
    return out


# revision 8
# speedup vs baseline: 1.8147x; 1.1387x over previous
"""Trainium2 Bass kernel for nn_MessageGNN (gnn_message_passing) — v2.

Destination-sharded edges across 8 cores (core k owns clauses
[k*50000,(k+1)*50000) and vars [k*12500,(k+1)*12500) plus every edge whose
destination falls in its slice), so segment sums are fully core-local.

One SPMD program (identical instruction stream on all 8 cores, per-core
data) dispatched with a single jit(shard_map) call — per-core schedule
constants (tiles per window, one-hot slice envelopes) are maxed/unioned
across cores on the host so the program is core-independent.

Per window of 1024 destinations:
  - src embeddings arrive as a host pre-gathered fp16 stream (edge-slot
    order), loaded feature-major with one HWDGE xbar-transpose DMA.
  - Edge MLP: per 128-edge tile, x^T / sat^T stationary against Wemb/Wsat,
    accumulating m[e,d] in PSUM; lrelu via ACT Prelu(alpha=.1) or DVE
    (0.1*x max x), alternating to balance engines.
  - Segment-mean via one-hot matmul: one-hot built by a two-scalar DVE op
    (iota == dst) * (1/cnt) at the per-tile envelope width, accumulated
    into a [128,1024] window PSUM as h^T.
  - Node MLP fused per 512-node half: feats+bias / ctx one-hot (host-built,
    DMA'd) / h / emb weight chunks; outputs transposed on PE (f16) and
    DMA'd out; phase-3 partial sums accumulate in a persistent PSUM tile.
The 64-row ctx update finishes on host from the per-core partial sums.
"""

import sys

sys.path.insert(0, "/opt/trn_rl_repo")

import numpy as np

NV, NC, NU, E, D = 100000, 400000, 64, 1200000, 128
M = 8
CS, VS = NC // M, NV // M
WIN = 1024
P = 128
PAD_DST = 1408.0

F16 = np.float16
F32 = np.float32


def _ceil(a, b):
    return -(-a // b)


def _prep_side(src, dst, sat, n_dst, tab16, we):
    """Edge-side prep: shared schedule + per-core slot tables.

    Returns (sched, percore) where sched is core-independent and percore[k]
    holds gs (pre-gathered src rows), satp ([5,S]), dw ([128,2*T_total]).
    """
    nwin = _ceil(n_dst, WIN)
    cores = []
    counts = np.zeros((M, nwin), np.int64)
    for k in range(M):
        base = k * n_dst
        mask = (dst >= base) & (dst < base + n_dst)
        es = np.nonzero(mask)[0]
        dstl = (dst[es] - base).astype(np.int64)
        order = np.argsort(dstl, kind="stable")
        es, dstl = es[order], dstl[order]
        counts[k] = np.bincount(dstl // WIN, minlength=nwin)
        cores.append((es, dstl))
    Tw = np.maximum(1, _ceil(counts.max(0), P)).astype(np.int64)
    toff = np.concatenate([[0], np.cumsum(Tw)])
    T_total = int(Tw.sum())
    S = T_total * P
    soff = toff * P

    slot_dst = np.full((M, S), -1, np.int64)
    slot_eid = np.full((M, S), -1, np.int64)
    for k, (es, dstl) in enumerate(cores):
        start = 0
        for wi in range(nwin):
            n = int(counts[k, wi])
            sl = slice(soff[wi], soff[wi] + n)
            slot_dst[k, sl] = dstl[start:start + n] - wi * WIN
            slot_eid[k, sl] = es[start:start + n]
            start += n

    # per-tile envelope (base/width across all cores) + 512-boundary slices
    tiles = []  # flat list over (window, tile): dict(base, width, slices)
    win_of_tile = np.repeat(np.arange(nwin), Tw)
    for ti in range(T_total):
        sl = slice(ti * P, (ti + 1) * P)
        dv = slot_dst[:, sl]
        real = dv >= 0
        if real.any():
            lo, hi = int(dv[real].min()), int(dv[real].max())
        else:
            lo, hi = 0, 0
        b0 = (lo // 32) * 32
        wd = _ceil(hi + 1 - b0, 32) * 32
        slices = []
        for h in (0, 1):
            s = max(b0, h * 512)
            e = min(b0 + wd, (h + 1) * 512)
            if s < e:
                slices.append((h, s - h * 512, s - b0, e - s))
        tiles.append(dict(base=b0, width=wd, slices=slices))

    # per-core tables
    percore = []
    base_of_slot = np.array([tiles[ti]["base"] for ti in range(T_total)],
                            np.int64).repeat(P)
    for k in range(M):
        dv, ev = slot_dst[k], slot_eid[k]
        real = dv >= 0
        dst_rel = np.full(S, PAD_DST, F32)
        dst_rel[real] = (dv[real] - base_of_slot[real]).astype(F32)
        wslot = np.zeros(S, F32)
        wslot[real] = we[dst[ev[real]]]
        dw = np.zeros((P, 2 * T_total), F32)
        dw[:, 0::2] = dst_rel.reshape(T_total, P).T
        dw[:, 1::2] = wslot.reshape(T_total, P).T
        satp = np.zeros((5, S), F16)
        satp[:4, real] = sat[ev[real]].T.astype(F16)
        satp[4, real] = 1.0
        gs = np.zeros((S, D), F16)
        gs[real] = tab16[src[ev[real]]]
        percore.append(dict(gs=gs, satp=satp, dw=dw))

    sched = dict(nwin=nwin, Tw=Tw.tolist(), toff=toff.tolist(),
                 soff=soff.tolist(), tiles=tiles, S=S, T_total=T_total,
                 Tmax=int(Tw.max()), Smax=int(Tw.max() * P))
    return sched, percore


def _prep_nodes(feats, emb16, ctx_ids, n_nodes, nwin):
    """Node tables for one core: fe [81, nwin*WIN], embl [128, nwin*WIN],
    ohj [128, nwin*512] (per-128-block ctx one-hot, u columns)."""
    Np = nwin * WIN
    fe = np.zeros((81, Np), F16)
    fe[ctx_ids, np.arange(n_nodes)] = 1.0          # rows 0:64 ctx one-hot
    fe[64:80, :n_nodes] = feats.T.astype(F16)      # rows 64:80 feats
    fe[80, :n_nodes] = 1.0                         # row 80 bias ones
    embl = np.zeros((P, Np), F16)
    embl[:, :n_nodes] = emb16.T
    nblk = nwin * 8
    ohj = np.zeros((P, nblk * 64), F16)
    node = np.arange(n_nodes)
    blk = node // P
    prow = node % P
    ohj[prow, blk * 64 + ctx_ids] = 1.0
    return fe, embl, ohj


def _build(sa, sb, nwinC, nwinV, repeat=1):
    import concourse.mybir as mybir
    import concourse.tile as tile
    from concourse import bacc
    from concourse.masks import make_identity

    f16, f32, i32 = mybir.dt.float16, mybir.dt.float32, mybir.dt.int32
    AF = mybir.ActivationFunctionType
    OP = mybir.AluOpType

    nc = bacc.Bacc("TRN2", target_bir_lowering=False, debug=False,
                   num_devices=1)
    io = {}

    def dram(name, shape, dt, kind="ExternalInput"):
        io[name] = nc.dram_tensor(name, list(shape), dt, kind=kind)
        return io[name]

    for side, s in (("A", sa), ("B", sb)):
        dram(f"gs{side}", [s["S"], D], f16)
        dram(f"satp{side}", [5, s["S"]], f16)
        dram(f"dw{side}", [P, 2 * s["T_total"]], f32)
        dram(f"Wemb{side}", [P, D], f16)
        dram(f"Wsat{side}", [5, D], f16)
    for sd, nwin in (("C", nwinC), ("V", nwinV)):
        dram(f"fe{sd}", [81, nwin * WIN], f16)
        dram(f"embl{sd}", [P, nwin * WIN], f16)
        dram(f"ohj{sd}", [P, nwin * 512], f16)
        dram(f"Wf{sd}", [P, D], f16)
        dram(f"Wh{sd}", [P, D], f16)
        dram(f"We{sd}", [P, D], f16)
        dram(f"ctxproj{sd}", [64, D], f16)
        dram(f"out{sd}", [nwin * 8, P, D], f16, kind="ExternalOutput")
    dram("acc", [P, P], f32, kind="ExternalOutput")

    with tile.TileContext(nc) as tc:
        with tc.tile_pool(name="const", bufs=1) as cpool, \
             tc.tile_pool(name="stage", bufs=2) as spool, \
             tc.tile_pool(name="meta", bufs=2) as mpool, \
             tc.tile_pool(name="work", bufs=3) as wpool, \
             tc.tile_pool(name="oh", bufs=10) as opool, \
             tc.tile_pool(name="node", bufs=2) as npool, \
             tc.tile_pool(name="psE", bufs=3, space="PSUM") as psE, \
             tc.tile_pool(name="psH", bufs=1, space="PSUM") as psH, \
             tc.tile_pool(name="psN", bufs=1, space="PSUM") as psN, \
             tc.tile_pool(name="psT", bufs=1, space="PSUM") as psT:

            identF = cpool.tile([P, P], f16)
            make_identity(nc, identF[:])
            iota_i = cpool.tile([P, WIN], i32)
            nc.gpsimd.iota(iota_i[:], pattern=[[1, WIN]], base=0,
                           channel_multiplier=0)
            iota16 = cpool.tile([P, WIN], f16)
            nc.vector.tensor_copy(iota16[:], iota_i[:])
            z128 = cpool.tile([1, P], f16)
            nc.gpsimd.memset(z128[:], 0.0)
            z512 = cpool.tile([1, 512], f16)
            nc.gpsimd.memset(z512[:], 0.0)

            wt = {}
            for nm in ("WembA", "WsatA", "WembB", "WsatB",
                       "WfC", "WhC", "WeC", "ctxprojC",
                       "WfV", "WhV", "WeV", "ctxprojV"):
                t = cpool.tile(list(io[nm].shape), f16, tag=nm)
                nc.sync.dma_start(t[:], io[nm][:])
                wt[nm] = t

            acc_sb = cpool.tile([P, P], f32, tag="accsbuf")

            Smax = max(sa["Smax"], sb["Smax"])
            Tmax = max(sa["Tmax"], sb["Tmax"])

            def issue_edge(side, s, wi):
                T = s["Tw"][wi]
                t0, s0 = s["toff"][wi], s["soff"][wi]
                slots = T * P
                stage = spool.tile([P, Smax], f16, tag="stage")
                nc.sync.dma_start(stage[:, :slots],
                                  io[f"gs{side}"][s0:s0 + slots, :],
                                  transpose=True)
                satp = mpool.tile([5, Smax], f16, tag="satp")
                nc.sync.dma_start(satp[:, :slots],
                                    io[f"satp{side}"][:, s0:s0 + slots])
                dw = mpool.tile([P, 2 * Tmax], f32, tag="dw")
                nc.sync.dma_start(dw[:, :2 * T],
                                    io[f"dw{side}"][:, 2 * t0:2 * (t0 + T)])
                return dict(stage=stage, satp=satp, dw=dw, T=T, t0=t0)

            def issue_node(sd, wi):
                cga = wi * WIN
                fe = npool.tile([81, WIN], f16, tag="fe")
                nc.sync.dma_start(fe[:], io[f"fe{sd}"][:, cga:cga + WIN])
                embl = npool.tile([P, WIN], f16, tag="embl")
                nc.sync.dma_start(embl[:], io[f"embl{sd}"][:, cga:cga + WIN])
                ohj = npool.tile([P, 512], f16, tag="ohj")
                nc.sync.dma_start(ohj[:],
                                    io[f"ohj{sd}"][:, wi * 512:(wi + 1) * 512])
                return dict(fe=fe, embl=embl, ohj=ohj)

            jobs = [("A", "C", sa, wi) for wi in range(nwinC)] + \
                   [("B", "V", sb, wi) for wi in range(nwinV)]
            jobs = jobs * repeat
            rep_starts = set(range(0, len(jobs), len(jobs) // repeat))
            pf_e = issue_edge(jobs[0][0], jobs[0][2], jobs[0][3])
            pf_n = issue_node(jobs[0][1], jobs[0][3])
            for ji, (side, sd, s, wi) in enumerate(jobs):
                if ji in rep_starts:
                    nc.vector.memset(acc_sb[:], 0.0)
                acccol = 0 if sd == "C" else 64
                cur_e, cur_n = pf_e, pf_n
                if ji + 1 < len(jobs):
                    nside, nsd, ns, nwi = jobs[ji + 1]
                    pf_e = issue_edge(nside, ns, nwi)
                    pf_n = issue_node(nsd, nwi)
                if True:
                    T, t0 = cur_e["T"], cur_e["t0"]
                    stage, satp, dw = (cur_e["stage"], cur_e["satp"],
                                       cur_e["dw"])
                    hps = [psH.tile([P, 512], f32, tag=f"h{i}", name=f"hps{i}")
                           for i in range(2)]
                    # last block touching each window half (for early hT copy)
                    bsplit = [0, 0]
                    for t in range(T):
                        for (h, _cw, _co, _ln) in s["tiles"][t0 + t]["slices"]:
                            bsplit[h] = max(bsplit[h], t // 4)
                    hT = npool.tile([P, WIN], f16, tag="hT")
                    def emit_oh(pend):
                        msb_p, b_p, nt_p, ohws_p = pend
                        for t4 in range(nt_p):
                            t = b_p * 4 + t4
                            tm = s["tiles"][t0 + t]
                            for (h, colw, colo, ln) in tm["slices"]:
                                nc.tensor.matmul(
                                    hps[h][:, colw:colw + ln],
                                    lhsT=msb_p[:, t4 * P:(t4 + 1) * P],
                                    rhs=ohws_p[t4][:, colo:colo + ln],
                                    start=False, stop=True,
                                    skip_group_check=True)

                    def emit_hcopy(b_done):
                        if b_done == bsplit[0]:
                            nc.vector.tensor_copy(hT[:, :512], hps[0][:])
                        if b_done == bsplit[1]:
                            nc.vector.tensor_copy(hT[:, 512:], hps[1][:])

                    pend = None
                    for b in range(_ceil(T, 4)):
                        nt = min(4, T - b * 4)
                        mps = psE.tile([P, 512], f32, tag="mps")
                        for t4 in range(nt):
                            t = b * 4 + t4
                            nc.tensor.matmul(
                                mps[:, t4 * P:(t4 + 1) * P],
                                lhsT=stage[:, t * P:(t + 1) * P],
                                rhs=wt[f"Wemb{side}"][:],
                                start=True, stop=False)
                            nc.tensor.matmul(
                                mps[:, t4 * P:(t4 + 1) * P],
                                lhsT=satp[:, t * P:(t + 1) * P],
                                rhs=wt[f"Wsat{side}"][:],
                                start=False, stop=True)
                        if b == 0:
                            for i in range(2):
                                nc.vector.memset(hps[i][:], 0.0)
                        ohws = []
                        for t4 in range(nt):
                            t = b * 4 + t4
                            tm = s["tiles"][t0 + t]
                            wd = tm["width"]
                            ohw = opool.tile([P, WIN], f16, tag="ohw")
                            nc.vector.tensor_scalar(
                                out=ohw[:, :wd], in0=iota16[:, :wd],
                                scalar1=dw[:, 2 * t:2 * t + 1],
                                scalar2=dw[:, 2 * t + 1:2 * t + 2],
                                op0=OP.is_equal, op1=OP.mult)
                            ohws.append(ohw)
                        msb = wpool.tile([P, 512], f16, tag="msb")
                        nc.scalar.activation(msb[:, :nt * P],
                                             mps[:, :nt * P],
                                             AF.Prelu, alpha=0.1)
                        if pend is not None:
                            emit_oh(pend)
                            emit_hcopy(pend[1])
                        pend = (msb, b, nt, ohws)
                    if pend is not None:
                        emit_oh(pend)
                        emit_hcopy(pend[1])

                    # ---- node phase for this window ----
                    fe, embl, ohj = cur_n["fe"], cur_n["embl"], cur_n["ohj"]
                    for g in (0, 1):
                        g0 = g * 512
                        nps = psN.tile([P, 512], f32, tag="nps")
                        nc.tensor.matmul(nps[:], lhsT=wt[f"Wf{sd}"][64:81, :],
                                         rhs=fe[64:81, g0:g0 + 512],
                                         start=True, stop=False)
                        nc.tensor.matmul(nps[:], lhsT=wt[f"ctxproj{sd}"][:],
                                         rhs=fe[:64, g0:g0 + 512],
                                         start=False, stop=False)
                        nc.tensor.matmul(nps[:], lhsT=wt[f"Wh{sd}"][:],
                                         rhs=hT[:, g0:g0 + 512],
                                         start=False, stop=False)
                        nc.tensor.matmul(nps[:], lhsT=wt[f"We{sd}"][:],
                                         rhs=embl[:, g0:g0 + 512],
                                         start=False, stop=True)
                        nsb = wpool.tile([P, 512], f16, tag="nsb")
                        nc.scalar.activation(nsb[:], nps[:], AF.Prelu,
                                             alpha=0.1)
                        tps = psT.tile([P, 512], f16, tag="tps")
                        for j in range(4):
                            nc.tensor.matmul(tps[:, j * P:(j + 1) * P],
                                             lhsT=nsb[:, j * P:(j + 1) * P],
                                             rhs=identF[:],
                                             is_transpose=True,
                                             skip_group_check=True)
                        osb = wpool.tile([P, 512], f16, tag="osb")
                        nc.vector.tensor_copy(osb[:], tps[:])
                        blk0 = (wi * 2 + g) * 4
                        nc.sync.dma_start(
                            io[f"out{sd}"][blk0:blk0 + 4, :, :]
                            .rearrange("j p d -> p j d"),
                            osb[:].rearrange("p (j d) -> p j d", j=4))
                        aps = psT.tile([P, 64], f32, tag="aps")
                        for j in range(4):
                            nc.tensor.matmul(
                                aps[:],
                                lhsT=osb[:, j * P:(j + 1) * P],
                                rhs=ohj[:, (g * 4 + j) * 64:(g * 4 + j + 1) * 64],
                                start=(j == 0), stop=(j == 3),
                                skip_group_check=True)
                        nc.vector.tensor_add(acc_sb[:, acccol:acccol + 64],
                                             acc_sb[:, acccol:acccol + 64],
                                             aps[:])
            nc.sync.dma_start(io["acc"][:], acc_sb[:])
    nc.compile()
    return nc


_timing_handles = []


def _run_spmd(nc, in_maps):
    """One jit(shard_map) dispatch running the SPMD program on 8 cores."""
    import jax
    from jax.sharding import Mesh, PartitionSpec, NamedSharding
    from jax.experimental.shard_map import shard_map
    from concourse.bass2jax import _bass_exec_p, install_neuronx_cc_hook
    import concourse.mybir as mybir

    install_neuronx_cc_hook()
    devs = jax.devices()[:M]

    in_names, out_names, out_avals, zero_outs = [], [], [], []
    pid_name = None
    for alloc in nc.m.functions[0].allocations:
        if not isinstance(alloc, mybir.MemoryLocationSet):
            continue
        name = alloc.memorylocations[0].name
        if alloc.kind == "ExternalInput":
            if name == "partition_id":
                pid_name = name
            else:
                in_names.append(name)
        elif alloc.kind == "ExternalOutput":
            shape = tuple(alloc.tensor_shape)
            dtype = mybir.dt.np(alloc.dtype)
            out_names.append(name)
            out_avals.append(jax.core.ShapedArray(shape, dtype))
            zero_outs.append(np.zeros(shape, dtype))
    n_params = len(in_names)
    all_names = list(in_names) + list(out_names)
    if pid_name:
        all_names.append(pid_name)

    def _body(*args, _oa=tuple(out_avals), _an=tuple(all_names),
              _on=tuple(out_names), _nc=nc):
        return tuple(_bass_exec_p.bind(
            *args, out_avals=_oa, in_names=_an, out_names=_on,
            lowering_input_output_aliases=(),
            sim_require_finite=True, sim_require_nnan=True, nc=_nc,
        ))

    mesh = Mesh(np.asarray(devs), ("core",))
    nops = n_params + len(zero_outs) + (1 if pid_name else 0)
    spec = PartitionSpec("core")
    fn = jax.jit(shard_map(_body, mesh=mesh, in_specs=(spec,) * nops,
                           out_specs=(spec,) * len(out_names),
                           check_rep=False), keep_unused=True)

    concat_in = [np.concatenate([np.asarray(m[nm]) for m in in_maps], axis=0)
                 for nm in in_names]
    concat_zeros = [np.zeros((M * z.shape[0], *z.shape[1:]), z.dtype)
                    for z in zero_outs]
    args = concat_in + concat_zeros
    if pid_name:
        args.append(np.arange(M, dtype=np.uint32).reshape(M, 1))

    def _body_for(nc_alt):
        def _body2(*args):
            return tuple(_bass_exec_p.bind(
                *args, out_avals=tuple(out_avals), in_names=tuple(all_names),
                out_names=tuple(out_names),
                lowering_input_output_aliases=(),
                sim_require_finite=True, sim_require_nnan=True, nc=nc_alt,
            ))
        return jax.jit(shard_map(_body2, mesh=mesh, in_specs=(spec,) * nops,
                                 out_specs=(spec,) * len(out_names),
                                 check_rep=False), keep_unused=True)

    def make_loop_fn(K):
        """jit that executes the kernel K times back-to-back on-device
        (scan carries the output buffers), for dispatch-free timing."""
        n_outs = len(out_names)

        def _body_k(*args):
            data = args[:n_params]
            zeros = tuple(args[n_params:n_params + n_outs])
            pid = args[-1] if pid_name else None

            def step(carry, _):
                operands = list(data) + list(carry)
                if pid is not None:
                    operands.append(pid)
                outs = _bass_exec_p.bind(
                    *operands, out_avals=tuple(out_avals),
                    in_names=tuple(all_names), out_names=tuple(out_names),
                    lowering_input_output_aliases=(),
                    sim_require_finite=True, sim_require_nnan=True, nc=nc)
                return tuple(outs), None

            carry, _ = jax.lax.scan(step, zeros, None, length=K)
            return carry

        return jax.jit(shard_map(_body_k, mesh=mesh, in_specs=(spec,) * nops,
                                 out_specs=(spec,) * len(out_names),
                                 check_rep=False), keep_unused=True)

    out_arrs = fn(*args)
    res = [np.asarray(o) for o in out_arrs]
    _timing_handles.append(dict(fn=fn, args=args, devs=devs, mesh=mesh,
                                make_loop_fn=make_loop_fn,
                                make_body=_body_for))
    return {nm: res[i].reshape(M, *out_avals[i].shape)
            for i, nm in enumerate(out_names)}


def kernel(**inputs):
    inp = {k: np.asarray(v) for k, v in inputs.items()}
    var_emb, clause_emb, ctx_emb = (inp["var_emb"], inp["clause_emb"],
                                    inp["ctx_emb"])
    nv, ncl, nu = var_emb.shape[0], clause_emb.shape[0], ctx_emb.shape[0]
    cs, vs = ncl // M, nv // M

    a_src = inp["assigns_src"].astype(np.int64)
    a_dst = inp["assigns_dst"].astype(np.int64)
    c_src = inp["contains_src"].astype(np.int64)
    c_dst = inp["contains_dst"].astype(np.int64)
    var_ctx = inp["var_ctx"].astype(np.int64)
    clause_ctx = inp["clause_ctx"].astype(np.int64)

    cnt_c = np.bincount(a_dst, minlength=ncl).astype(F32)
    cnt_v = np.bincount(c_dst, minlength=nv).astype(F32)
    we_c = 1.0 / np.maximum(cnt_c, 1.0)
    we_v = 1.0 / np.maximum(cnt_v, 1.0)

    var16 = var_emb.astype(F16)
    clause16 = clause_emb.astype(F16)

    W_vc, b_vc = inp["W_vc"].astype(F32), inp["b_vc"].astype(F32)
    W_cv, b_cv = inp["W_cv"].astype(F32), inp["b_cv"].astype(F32)

    sa, pcA = _prep_side(a_src, a_dst, inp["edge_sat_vc"], cs, var16, we_c)
    sb, pcB = _prep_side(c_src, c_dst, inp["edge_sat_cv"], vs, clause16, we_v)
    nwinC, nwinV = sa["nwin"], sb["nwin"]

    def node_w(Wn, bn):
        Wn, bn = Wn.astype(F32), bn.astype(F32)
        Wf = np.zeros((P, D), F16)
        Wf[64:80] = Wn[:16].astype(F16)
        Wf[80] = bn.astype(F16)
        Wh = np.ascontiguousarray(Wn[16:16 + D]).astype(F16)
        ctxproj = (ctx_emb.astype(F32) @ Wn[16 + D:16 + 2 * D]).astype(F16)
        We = np.ascontiguousarray(Wn[16 + 2 * D:16 + 3 * D]).astype(F16)
        return Wf, Wh, ctxproj, We

    WfC, WhC, ctxprojC, WeC = node_w(inp["W_c"], inp["b_c"])
    WfV, WhV, ctxprojV, WeV = node_w(inp["W_v"], inp["b_v"])

    common = dict(
        WembA=np.ascontiguousarray(W_vc[4:4 + D]).astype(F16),
        WsatA=np.vstack([W_vc[:4], b_vc[None, :]]).astype(F16),
        WembB=np.ascontiguousarray(W_cv[4:4 + D]).astype(F16),
        WsatB=np.vstack([W_cv[:4], b_cv[None, :]]).astype(F16),
        WfC=WfC, WhC=WhC, WeC=WeC, ctxprojC=ctxprojC,
        WfV=WfV, WhV=WhV, WeV=WeV, ctxprojV=ctxprojV,
    )

    in_maps = []
    for k in range(M):
        feC, emC, ohjC = _prep_nodes(
            inp["clause_feats"][k * cs:(k + 1) * cs],
            clause16[k * cs:(k + 1) * cs],
            clause_ctx[k * cs:(k + 1) * cs], cs, nwinC)
        feV, emV, ohjV = _prep_nodes(
            inp["var_feats"][k * vs:(k + 1) * vs],
            var16[k * vs:(k + 1) * vs],
            var_ctx[k * vs:(k + 1) * vs], vs, nwinV)
        in_maps.append(dict(
            gsA=pcA[k]["gs"], satpA=pcA[k]["satp"], dwA=pcA[k]["dw"],
            gsB=pcB[k]["gs"], satpB=pcB[k]["satp"], dwB=pcB[k]["dw"],
            feC=feC, emblC=emC, ohjC=ohjC,
            feV=feV, emblV=emV, ohjV=ohjV,
            **common,
        ))

    nc = _build(sa, sb, nwinC, nwinV)
    res = _run_spmd(nc, in_maps)
    _timing_handles[-1]["make_repeat"] = (
        lambda r: _timing_handles[-1]["make_body"](
            _build(sa, sb, nwinC, nwinV, repeat=r)))

    new_clause = res["outC"].reshape(M, -1, D)[:, :cs].reshape(ncl, D)
    new_var = res["outV"].reshape(M, -1, D)[:, :vs].reshape(nv, D)
    acc = res["acc"].sum(0)  # [128 d, 128] cols 0:64 C, 64:128 V
    accC, accV = acc[:, :64], acc[:, 64:]

    cnt_cu = np.bincount(clause_ctx, minlength=nu).astype(F32)
    cnt_vu = np.bincount(var_ctx, minlength=nu).astype(F32)
    c_ctx = (accC / np.maximum(cnt_cu, 1.0)[None, :]).T
    v_ctx = (accV / np.maximum(cnt_vu, 1.0)[None, :]).T
    zu = np.concatenate([inp["ctx_feats"].astype(F32), c_ctx, v_ctx,
                         ctx_emb.astype(F32)], 1) @ inp["W_u"].astype(F32) \
        + inp["b_u"].astype(F32)
    new_ctx = np.where(zu >= 0, zu, 0.1 * zu).astype(F32)

    out = np.empty((ncl + nv + nu, D), F32)
    out[:ncl] = new_clause.astype(F32)
    out[ncl:ncl + nv] = new_var.astype(F32)
    out[ncl + nv:] = new_ctx
    return out


# revision 9
# speedup vs baseline: 1.8772x; 1.0344x over previous
"""Trainium2 Bass kernel for nn_MessageGNN (gnn_message_passing) — v2.

Destination-sharded edges across 8 cores (core k owns clauses
[k*50000,(k+1)*50000) and vars [k*12500,(k+1)*12500) plus every edge whose
destination falls in its slice), so segment sums are fully core-local.

One SPMD program (identical instruction stream on all 8 cores, per-core
data) dispatched with a single jit(shard_map) call — per-core schedule
constants (tiles per window, one-hot slice envelopes) are maxed/unioned
across cores on the host so the program is core-independent.

Per window of 1024 destinations:
  - src embeddings arrive as a host pre-gathered fp16 stream (edge-slot
    order), loaded feature-major with one HWDGE xbar-transpose DMA.
  - Edge MLP: per 128-edge tile, x^T / sat^T stationary against Wemb/Wsat,
    accumulating m[e,d] in PSUM; lrelu via ACT Prelu(alpha=.1) or DVE
    (0.1*x max x), alternating to balance engines.
  - Segment-mean via one-hot matmul: one-hot built by a two-scalar DVE op
    (iota == dst) * (1/cnt) at the per-tile envelope width, accumulated
    into a [128,1024] window PSUM as h^T.
  - Node MLP fused per 512-node half: feats+bias / ctx one-hot (host-built,
    DMA'd) / h / emb weight chunks; outputs transposed on PE (f16) and
    DMA'd out; phase-3 partial sums accumulate in a persistent PSUM tile.
The 64-row ctx update finishes on host from the per-core partial sums.
"""

import sys

sys.path.insert(0, "/opt/trn_rl_repo")

import numpy as np

NV, NC, NU, E, D = 100000, 400000, 64, 1200000, 128
M = 8
CS, VS = NC // M, NV // M
WIN = 1024
P = 128
PAD_DST = 1408.0

F16 = np.float16
F32 = np.float32


def _ceil(a, b):
    return -(-a // b)


def _prep_side(src, dst, sat, n_dst, tab16, we):
    """Edge-side prep: shared schedule + per-core slot tables.

    Returns (sched, percore) where sched is core-independent and percore[k]
    holds gs (pre-gathered src rows), satp ([5,S]), dw ([128,2*T_total]).
    """
    nwin = _ceil(n_dst, WIN)
    cores = []
    counts = np.zeros((M, nwin), np.int64)
    for k in range(M):
        base = k * n_dst
        mask = (dst >= base) & (dst < base + n_dst)
        es = np.nonzero(mask)[0]
        dstl = (dst[es] - base).astype(np.int64)
        order = np.argsort(dstl, kind="stable")
        es, dstl = es[order], dstl[order]
        counts[k] = np.bincount(dstl // WIN, minlength=nwin)
        cores.append((es, dstl))
    Tw = np.maximum(1, _ceil(counts.max(0), P)).astype(np.int64)
    toff = np.concatenate([[0], np.cumsum(Tw)])
    T_total = int(Tw.sum())
    S = T_total * P
    soff = toff * P

    slot_dst = np.full((M, S), -1, np.int64)
    slot_eid = np.full((M, S), -1, np.int64)
    for k, (es, dstl) in enumerate(cores):
        start = 0
        for wi in range(nwin):
            n = int(counts[k, wi])
            sl = slice(soff[wi], soff[wi] + n)
            slot_dst[k, sl] = dstl[start:start + n] - wi * WIN
            slot_eid[k, sl] = es[start:start + n]
            start += n

    # per-tile envelope (base/width across all cores) + 512-boundary slices
    tiles = []  # flat list over (window, tile): dict(base, width, slices)
    win_of_tile = np.repeat(np.arange(nwin), Tw)
    for ti in range(T_total):
        sl = slice(ti * P, (ti + 1) * P)
        dv = slot_dst[:, sl]
        real = dv >= 0
        if real.any():
            lo, hi = int(dv[real].min()), int(dv[real].max())
        else:
            lo, hi = 0, 0
        b0 = (lo // 32) * 32
        wd = _ceil(hi + 1 - b0, 32) * 32
        slices = []
        for h in (0, 1):
            s = max(b0, h * 512)
            e = min(b0 + wd, (h + 1) * 512)
            if s < e:
                slices.append((h, s - h * 512, s - b0, e - s))
        tiles.append(dict(base=b0, width=wd, slices=slices))

    # per-core tables
    percore = []
    base_of_slot = np.array([tiles[ti]["base"] for ti in range(T_total)],
                            np.int64).repeat(P)
    for k in range(M):
        dv, ev = slot_dst[k], slot_eid[k]
        real = dv >= 0
        dst_rel = np.full(S, PAD_DST, F32)
        dst_rel[real] = (dv[real] - base_of_slot[real]).astype(F32)
        wslot = np.zeros(S, F32)
        wslot[real] = we[dst[ev[real]]]
        dw = np.zeros((P, 2 * T_total), F32)
        dw[:, 0::2] = dst_rel.reshape(T_total, P).T
        dw[:, 1::2] = wslot.reshape(T_total, P).T
        satp = np.zeros((5, S), F16)
        satp[:4, real] = sat[ev[real]].T.astype(F16)
        satp[4, real] = 1.0
        gs = np.zeros((S, D), F16)
        gs[real] = tab16[src[ev[real]]]
        percore.append(dict(gs=gs, satp=satp, dw=dw))

    sched = dict(nwin=nwin, Tw=Tw.tolist(), toff=toff.tolist(),
                 soff=soff.tolist(), tiles=tiles, S=S, T_total=T_total,
                 Tmax=int(Tw.max()), Smax=int(Tw.max() * P))
    return sched, percore


def _prep_nodes(feats, emb16, ctx_ids, n_nodes, nwin):
    """Node tables for one core: fe [81, nwin*WIN], embl [128, nwin*WIN],
    ohj [128, nwin*512] (per-128-block ctx one-hot, u columns)."""
    Np = nwin * WIN
    fe = np.zeros((81, Np), F16)
    fe[ctx_ids, np.arange(n_nodes)] = 1.0          # rows 0:64 ctx one-hot
    fe[64:80, :n_nodes] = feats.T.astype(F16)      # rows 64:80 feats
    fe[80, :n_nodes] = 1.0                         # row 80 bias ones
    embl = np.zeros((P, Np), F16)
    embl[:, :n_nodes] = emb16.T
    nblk = nwin * 8
    ohj = np.zeros((P, nblk * 64), F16)
    node = np.arange(n_nodes)
    blk = node // P
    prow = node % P
    ohj[prow, blk * 64 + ctx_ids] = 1.0
    return fe, embl, ohj


def _build(sa, sb, nwinC, nwinV, repeat=1):
    import concourse.mybir as mybir
    import concourse.tile as tile
    from concourse import bacc
    from concourse.masks import make_identity

    f16, f32, i32 = mybir.dt.float16, mybir.dt.float32, mybir.dt.int32
    AF = mybir.ActivationFunctionType
    OP = mybir.AluOpType

    nc = bacc.Bacc("TRN2", target_bir_lowering=False, debug=False,
                   num_devices=1)
    io = {}

    def dram(name, shape, dt, kind="ExternalInput"):
        io[name] = nc.dram_tensor(name, list(shape), dt, kind=kind)
        return io[name]

    for side, s in (("A", sa), ("B", sb)):
        dram(f"gs{side}", [s["S"], D], f16)
        dram(f"satp{side}", [5, s["S"]], f16)
        dram(f"dw{side}", [P, 2 * s["T_total"]], f32)
        dram(f"Wemb{side}", [P, D], f16)
        dram(f"Wsat{side}", [5, D], f16)
    for sd, nwin in (("C", nwinC), ("V", nwinV)):
        dram(f"fe{sd}", [81, nwin * WIN], f16)
        dram(f"embl{sd}", [P, nwin * WIN], f16)
        dram(f"ohj{sd}", [P, nwin * 512], f16)
        dram(f"Wf{sd}", [P, D], f16)
        dram(f"Wh{sd}", [P, D], f16)
        dram(f"We{sd}", [P, D], f16)
        dram(f"ctxproj{sd}", [64, D], f16)
        dram(f"out{sd}", [nwin * 8, P, D], f16, kind="ExternalOutput")
    dram("acc", [P, P], f32, kind="ExternalOutput")

    with tile.TileContext(nc) as tc:
        with tc.tile_pool(name="const", bufs=1) as cpool, \
             tc.tile_pool(name="stage", bufs=2) as spool, \
             tc.tile_pool(name="meta", bufs=2) as mpool, \
             tc.tile_pool(name="work", bufs=4) as wpool, \
             tc.tile_pool(name="oh", bufs=12) as opool, \
             tc.tile_pool(name="node", bufs=2) as npool, \
             tc.tile_pool(name="psE", bufs=3, space="PSUM") as psE, \
             tc.tile_pool(name="psH", bufs=1, space="PSUM") as psH, \
             tc.tile_pool(name="psN", bufs=2, space="PSUM") as psN, \
             tc.tile_pool(name="psT", bufs=1, space="PSUM") as psT:

            identF = cpool.tile([P, P], f16)
            make_identity(nc, identF[:])
            iota_i = cpool.tile([P, WIN], i32)
            nc.gpsimd.iota(iota_i[:], pattern=[[1, WIN]], base=0,
                           channel_multiplier=0)
            iota16 = cpool.tile([P, WIN], f16)
            nc.vector.tensor_copy(iota16[:], iota_i[:])
            z128 = cpool.tile([1, P], f16)
            nc.gpsimd.memset(z128[:], 0.0)
            z512 = cpool.tile([1, 512], f16)
            nc.gpsimd.memset(z512[:], 0.0)

            wt = {}
            for nm in ("WembA", "WsatA", "WembB", "WsatB",
                       "WfC", "WhC", "WeC", "ctxprojC",
                       "WfV", "WhV", "WeV", "ctxprojV"):
                t = cpool.tile(list(io[nm].shape), f16, tag=nm)
                nc.sync.dma_start(t[:], io[nm][:])
                wt[nm] = t

            acc_sb = cpool.tile([P, P], f32, tag="accsbuf")

            Smax = max(sa["Smax"], sb["Smax"])
            Tmax = max(sa["Tmax"], sb["Tmax"])

            def issue_edge(side, s, wi):
                T = s["Tw"][wi]
                t0, s0 = s["toff"][wi], s["soff"][wi]
                slots = T * P
                stage = spool.tile([P, Smax], f16, tag="stage")
                nc.sync.dma_start(stage[:, :slots],
                                  io[f"gs{side}"][s0:s0 + slots, :],
                                  transpose=True)
                satp = mpool.tile([5, Smax], f16, tag="satp")
                nc.sync.dma_start(satp[:, :slots],
                                    io[f"satp{side}"][:, s0:s0 + slots])
                dw = mpool.tile([P, 2 * Tmax], f32, tag="dw")
                nc.sync.dma_start(dw[:, :2 * T],
                                    io[f"dw{side}"][:, 2 * t0:2 * (t0 + T)])
                return dict(stage=stage, satp=satp, dw=dw, T=T, t0=t0)

            def issue_node(sd, wi):
                cga = wi * WIN
                fe = npool.tile([81, WIN], f16, tag="fe")
                nc.sync.dma_start(fe[:], io[f"fe{sd}"][:, cga:cga + WIN])
                embl = npool.tile([P, WIN], f16, tag="embl")
                nc.sync.dma_start(embl[:], io[f"embl{sd}"][:, cga:cga + WIN])
                ohj = npool.tile([P, 512], f16, tag="ohj")
                nc.sync.dma_start(ohj[:],
                                    io[f"ohj{sd}"][:, wi * 512:(wi + 1) * 512])
                return dict(fe=fe, embl=embl, ohj=ohj)

            jobs = [("A", "C", sa, wi) for wi in range(nwinC)] + \
                   [("B", "V", sb, wi) for wi in range(nwinV)]
            jobs = jobs * repeat
            rep_starts = set(range(0, len(jobs), len(jobs) // repeat))
            pf_e = issue_edge(jobs[0][0], jobs[0][2], jobs[0][3])
            pf_n = issue_node(jobs[0][1], jobs[0][3])
            for ji, (side, sd, s, wi) in enumerate(jobs):
                if ji in rep_starts:
                    nc.vector.memset(acc_sb[:], 0.0)
                acccol = 0 if sd == "C" else 64
                cur_e, cur_n = pf_e, pf_n
                if ji + 1 < len(jobs):
                    nside, nsd, ns, nwi = jobs[ji + 1]
                    pf_e = issue_edge(nside, ns, nwi)
                    pf_n = issue_node(nsd, nwi)
                if True:
                    T, t0 = cur_e["T"], cur_e["t0"]
                    stage, satp, dw = (cur_e["stage"], cur_e["satp"],
                                       cur_e["dw"])
                    hps = [psH.tile([P, 512], f32, tag=f"h{i}", name=f"hps{i}")
                           for i in range(2)]
                    # last block touching each window half (for early hT copy)
                    bsplit = [0, 0]
                    for t in range(T):
                        for (h, _cw, _co, _ln) in s["tiles"][t0 + t]["slices"]:
                            bsplit[h] = max(bsplit[h], t // 4)
                    hT = npool.tile([P, WIN], f16, tag="hT")
                    def emit_oh(pend):
                        msb_p, b_p, nt_p, ohws_p = pend
                        for t4 in range(nt_p):
                            t = b_p * 4 + t4
                            tm = s["tiles"][t0 + t]
                            for (h, colw, colo, ln) in tm["slices"]:
                                nc.tensor.matmul(
                                    hps[h][:, colw:colw + ln],
                                    lhsT=msb_p[:, t4 * P:(t4 + 1) * P],
                                    rhs=ohws_p[t4][:, colo:colo + ln],
                                    start=False, stop=True,
                                    skip_group_check=True)

                    def emit_hcopy(b_done):
                        if b_done == bsplit[0]:
                            nc.vector.tensor_copy(hT[:, :512], hps[0][:])
                        if b_done == bsplit[1]:
                            nc.vector.tensor_copy(hT[:, 512:], hps[1][:])

                    pend = None
                    for b in range(_ceil(T, 4)):
                        nt = min(4, T - b * 4)
                        mps = psE.tile([P, 512], f32, tag="mps")
                        for t4 in range(nt):
                            t = b * 4 + t4
                            nc.tensor.matmul(
                                mps[:, t4 * P:(t4 + 1) * P],
                                lhsT=stage[:, t * P:(t + 1) * P],
                                rhs=wt[f"Wemb{side}"][:],
                                start=True, stop=False)
                            nc.tensor.matmul(
                                mps[:, t4 * P:(t4 + 1) * P],
                                lhsT=satp[:, t * P:(t + 1) * P],
                                rhs=wt[f"Wsat{side}"][:],
                                start=False, stop=True)
                        if b == 0:
                            for i in range(2):
                                nc.vector.memset(hps[i][:], 0.0)
                        ohws = []
                        for t4 in range(nt):
                            t = b * 4 + t4
                            tm = s["tiles"][t0 + t]
                            wd = tm["width"]
                            ohw = opool.tile([P, WIN], f16, tag="ohw")
                            nc.vector.tensor_scalar(
                                out=ohw[:, :wd], in0=iota16[:, :wd],
                                scalar1=dw[:, 2 * t:2 * t + 1],
                                scalar2=dw[:, 2 * t + 1:2 * t + 2],
                                op0=OP.is_equal, op1=OP.mult)
                            ohws.append(ohw)
                        msb = wpool.tile([P, 512], f16, tag="msb")
                        nc.scalar.activation(msb[:, :nt * P],
                                             mps[:, :nt * P],
                                             AF.Prelu, alpha=0.1)
                        if pend is not None:
                            emit_oh(pend)
                            emit_hcopy(pend[1])
                        pend = (msb, b, nt, ohws)
                    if pend is not None:
                        emit_oh(pend)
                        emit_hcopy(pend[1])

                    # ---- node phase for this window ----
                    fe, embl, ohj = cur_n["fe"], cur_n["embl"], cur_n["ohj"]
                    for g in (0, 1):
                        g0 = g * 512
                        nps = psN.tile([P, 512], f32, tag="nps")
                        nc.tensor.matmul(nps[:], lhsT=wt[f"Wf{sd}"][64:81, :],
                                         rhs=fe[64:81, g0:g0 + 512],
                                         start=True, stop=False)
                        nc.tensor.matmul(nps[:], lhsT=wt[f"ctxproj{sd}"][:],
                                         rhs=fe[:64, g0:g0 + 512],
                                         start=False, stop=False)
                        nc.tensor.matmul(nps[:], lhsT=wt[f"Wh{sd}"][:],
                                         rhs=hT[:, g0:g0 + 512],
                                         start=False, stop=False)
                        nc.tensor.matmul(nps[:], lhsT=wt[f"We{sd}"][:],
                                         rhs=embl[:, g0:g0 + 512],
                                         start=False, stop=True)
                        nsb = wpool.tile([P, 512], f16, tag="nsb")
                        nc.scalar.activation(nsb[:], nps[:], AF.Prelu,
                                             alpha=0.1)
                        tps = psT.tile([P, 512], f16, tag="tps")
                        for j in range(4):
                            nc.tensor.matmul(tps[:, j * P:(j + 1) * P],
                                             lhsT=nsb[:, j * P:(j + 1) * P],
                                             rhs=identF[:],
                                             is_transpose=True,
                                             skip_group_check=True)
                        osb = wpool.tile([P, 512], f16, tag="osb")
                        nc.vector.tensor_copy(osb[:], tps[:])
                        blk0 = (wi * 2 + g) * 4
                        nc.sync.dma_start(
                            io[f"out{sd}"][blk0:blk0 + 4, :, :]
                            .rearrange("j p d -> p j d"),
                            osb[:].rearrange("p (j d) -> p j d", j=4))
                        # phase-3 partials reuse nps cols 448:512 (dead
                        # after the Prelu read) — saves a PSUM bank
                        for j in range(4):
                            nc.tensor.matmul(
                                nps[:, 448:512],
                                lhsT=osb[:, j * P:(j + 1) * P],
                                rhs=ohj[:, (g * 4 + j) * 64:(g * 4 + j + 1) * 64],
                                start=(j == 0), stop=(j == 3),
                                skip_group_check=True)
                        nc.vector.tensor_add(acc_sb[:, acccol:acccol + 64],
                                             acc_sb[:, acccol:acccol + 64],
                                             nps[:, 448:512])
            nc.sync.dma_start(io["acc"][:], acc_sb[:])
    nc.compile()
    return nc


_timing_handles = []


def _run_spmd(nc, in_maps):
    """One jit(shard_map) dispatch running the SPMD program on 8 cores."""
    import jax
    from jax.sharding import Mesh, PartitionSpec, NamedSharding
    from jax.experimental.shard_map import shard_map
    from concourse.bass2jax import _bass_exec_p, install_neuronx_cc_hook
    import concourse.mybir as mybir

    install_neuronx_cc_hook()
    devs = jax.devices()[:M]

    in_names, out_names, out_avals, zero_outs = [], [], [], []
    pid_name = None
    for alloc in nc.m.functions[0].allocations:
        if not isinstance(alloc, mybir.MemoryLocationSet):
            continue
        name = alloc.memorylocations[0].name
        if alloc.kind == "ExternalInput":
            if name == "partition_id":
                pid_name = name
            else:
                in_names.append(name)
        elif alloc.kind == "ExternalOutput":
            shape = tuple(alloc.tensor_shape)
            dtype = mybir.dt.np(alloc.dtype)
            out_names.append(name)
            out_avals.append(jax.core.ShapedArray(shape, dtype))
            zero_outs.append(np.zeros(shape, dtype))
    n_params = len(in_names)
    all_names = list(in_names) + list(out_names)
    if pid_name:
        all_names.append(pid_name)

    def _body(*args, _oa=tuple(out_avals), _an=tuple(all_names),
              _on=tuple(out_names), _nc=nc):
        return tuple(_bass_exec_p.bind(
            *args, out_avals=_oa, in_names=_an, out_names=_on,
            lowering_input_output_aliases=(),
            sim_require_finite=True, sim_require_nnan=True, nc=_nc,
        ))

    mesh = Mesh(np.asarray(devs), ("core",))
    nops = n_params + len(zero_outs) + (1 if pid_name else 0)
    spec = PartitionSpec("core")
    fn = jax.jit(shard_map(_body, mesh=mesh, in_specs=(spec,) * nops,
                           out_specs=(spec,) * len(out_names),
                           check_rep=False), keep_unused=True)

    concat_in = [np.concatenate([np.asarray(m[nm]) for m in in_maps], axis=0)
                 for nm in in_names]
    concat_zeros = [np.zeros((M * z.shape[0], *z.shape[1:]), z.dtype)
                    for z in zero_outs]
    args = concat_in + concat_zeros
    if pid_name:
        args.append(np.arange(M, dtype=np.uint32).reshape(M, 1))

    def _body_for(nc_alt):
        def _body2(*args):
            return tuple(_bass_exec_p.bind(
                *args, out_avals=tuple(out_avals), in_names=tuple(all_names),
                out_names=tuple(out_names),
                lowering_input_output_aliases=(),
                sim_require_finite=True, sim_require_nnan=True, nc=nc_alt,
            ))
        return jax.jit(shard_map(_body2, mesh=mesh, in_specs=(spec,) * nops,
                                 out_specs=(spec,) * len(out_names),
                                 check_rep=False), keep_unused=True)

    def make_loop_fn(K):
        """jit that executes the kernel K times back-to-back on-device
        (scan carries the output buffers), for dispatch-free timing."""
        n_outs = len(out_names)

        def _body_k(*args):
            data = args[:n_params]
            zeros = tuple(args[n_params:n_params + n_outs])
            pid = args[-1] if pid_name else None

            def step(carry, _):
                operands = list(data) + list(carry)
                if pid is not None:
                    operands.append(pid)
                outs = _bass_exec_p.bind(
                    *operands, out_avals=tuple(out_avals),
                    in_names=tuple(all_names), out_names=tuple(out_names),
                    lowering_input_output_aliases=(),
                    sim_require_finite=True, sim_require_nnan=True, nc=nc)
                return tuple(outs), None

            carry, _ = jax.lax.scan(step, zeros, None, length=K)
            return carry

        return jax.jit(shard_map(_body_k, mesh=mesh, in_specs=(spec,) * nops,
                                 out_specs=(spec,) * len(out_names),
                                 check_rep=False), keep_unused=True)

    out_arrs = fn(*args)
    res = [np.asarray(o) for o in out_arrs]
    _timing_handles.append(dict(fn=fn, args=args, devs=devs, mesh=mesh,
                                make_loop_fn=make_loop_fn,
                                make_body=_body_for))
    return {nm: res[i].reshape(M, *out_avals[i].shape)
            for i, nm in enumerate(out_names)}


def kernel(**inputs):
    inp = {k: np.asarray(v) for k, v in inputs.items()}
    var_emb, clause_emb, ctx_emb = (inp["var_emb"], inp["clause_emb"],
                                    inp["ctx_emb"])
    nv, ncl, nu = var_emb.shape[0], clause_emb.shape[0], ctx_emb.shape[0]
    cs, vs = ncl // M, nv // M

    a_src = inp["assigns_src"].astype(np.int64)
    a_dst = inp["assigns_dst"].astype(np.int64)
    c_src = inp["contains_src"].astype(np.int64)
    c_dst = inp["contains_dst"].astype(np.int64)
    var_ctx = inp["var_ctx"].astype(np.int64)
    clause_ctx = inp["clause_ctx"].astype(np.int64)

    cnt_c = np.bincount(a_dst, minlength=ncl).astype(F32)
    cnt_v = np.bincount(c_dst, minlength=nv).astype(F32)
    we_c = 1.0 / np.maximum(cnt_c, 1.0)
    we_v = 1.0 / np.maximum(cnt_v, 1.0)

    var16 = var_emb.astype(F16)
    clause16 = clause_emb.astype(F16)

    W_vc, b_vc = inp["W_vc"].astype(F32), inp["b_vc"].astype(F32)
    W_cv, b_cv = inp["W_cv"].astype(F32), inp["b_cv"].astype(F32)

    sa, pcA = _prep_side(a_src, a_dst, inp["edge_sat_vc"], cs, var16, we_c)
    sb, pcB = _prep_side(c_src, c_dst, inp["edge_sat_cv"], vs, clause16, we_v)
    nwinC, nwinV = sa["nwin"], sb["nwin"]

    def node_w(Wn, bn):
        Wn, bn = Wn.astype(F32), bn.astype(F32)
        Wf = np.zeros((P, D), F16)
        Wf[64:80] = Wn[:16].astype(F16)
        Wf[80] = bn.astype(F16)
        Wh = np.ascontiguousarray(Wn[16:16 + D]).astype(F16)
        ctxproj = (ctx_emb.astype(F32) @ Wn[16 + D:16 + 2 * D]).astype(F16)
        We = np.ascontiguousarray(Wn[16 + 2 * D:16 + 3 * D]).astype(F16)
        return Wf, Wh, ctxproj, We

    WfC, WhC, ctxprojC, WeC = node_w(inp["W_c"], inp["b_c"])
    WfV, WhV, ctxprojV, WeV = node_w(inp["W_v"], inp["b_v"])

    common = dict(
        WembA=np.ascontiguousarray(W_vc[4:4 + D]).astype(F16),
        WsatA=np.vstack([W_vc[:4], b_vc[None, :]]).astype(F16),
        WembB=np.ascontiguousarray(W_cv[4:4 + D]).astype(F16),
        WsatB=np.vstack([W_cv[:4], b_cv[None, :]]).astype(F16),
        WfC=WfC, WhC=WhC, WeC=WeC, ctxprojC=ctxprojC,
        WfV=WfV, WhV=WhV, WeV=WeV, ctxprojV=ctxprojV,
    )

    in_maps = []
    for k in range(M):
        feC, emC, ohjC = _prep_nodes(
            inp["clause_feats"][k * cs:(k + 1) * cs],
            clause16[k * cs:(k + 1) * cs],
            clause_ctx[k * cs:(k + 1) * cs], cs, nwinC)
        feV, emV, ohjV = _prep_nodes(
            inp["var_feats"][k * vs:(k + 1) * vs],
            var16[k * vs:(k + 1) * vs],
            var_ctx[k * vs:(k + 1) * vs], vs, nwinV)
        in_maps.append(dict(
            gsA=pcA[k]["gs"], satpA=pcA[k]["satp"], dwA=pcA[k]["dw"],
            gsB=pcB[k]["gs"], satpB=pcB[k]["satp"], dwB=pcB[k]["dw"],
            feC=feC, emblC=emC, ohjC=ohjC,
            feV=feV, emblV=emV, ohjV=ohjV,
            **common,
        ))

    nc = _build(sa, sb, nwinC, nwinV)
    res = _run_spmd(nc, in_maps)
    _timing_handles[-1]["make_repeat"] = (
        lambda r: _timing_handles[-1]["make_body"](
            _build(sa, sb, nwinC, nwinV, repeat=r)))

    new_clause = res["outC"].reshape(M, -1, D)[:, :cs].reshape(ncl, D)
    new_var = res["outV"].reshape(M, -1, D)[:, :vs].reshape(nv, D)
    acc = res["acc"].sum(0)  # [128 d, 128] cols 0:64 C, 64:128 V
    accC, accV = acc[:, :64], acc[:, 64:]

    cnt_cu = np.bincount(clause_ctx, minlength=nu).astype(F32)
    cnt_vu = np.bincount(var_ctx, minlength=nu).astype(F32)
    c_ctx = (accC / np.maximum(cnt_cu, 1.0)[None, :]).T
    v_ctx = (accV / np.maximum(cnt_vu, 1.0)[None, :]).T
    zu = np.concatenate([inp["ctx_feats"].astype(F32), c_ctx, v_ctx,
                         ctx_emb.astype(F32)], 1) @ inp["W_u"].astype(F32) \
        + inp["b_u"].astype(F32)
    new_ctx = np.where(zu >= 0, zu, 0.1 * zu).astype(F32)

    out = np.empty((ncl + nv + nu, D), F32)
    out[:ncl] = new_clause.astype(F32)
    out[ncl:ncl + nv] = new_var.astype(F32)
    out[ncl + nv:] = new_ctx
    return out
